# revision 34
# baseline (speedup 1.0000x reference)
"""Trainium2 Bass kernel: causal multi-head attention with interleaved RoPE.

Problem shapes (hardcoded): x [2, 2048, 1024], 16 heads of dk=64.
Sharding: 8 cores = 2 batches x 4 head-groups (4 heads each). Each core
computes its head-slice Q/K/V projections, RoPE, causal attention, and a
partial output through its Wo row-slice; the host sums the 4 partials per
batch and adds bo.

RoPE trick: attention scores are invariant to any permutation of the dk
axis applied to both Q and K, so the Wq/Wk columns are permuted on the host
into a "quadrant half-split" layout where each rotation pair partner sits
exactly 16 partitions away inside the same 32-partition quadrant. The DVE
stream_shuffle (a per-quadrant 32-way permute) then produces the swapped
operand, and RoPE becomes: rot = q * cosT + shuffle(q) * sinT with
host-precomputed tables (sinT carries the sign).

Schedule: one interleaved instruction stream. Projections for seq-block
j+1 and the Wo output projection for block j-1 are emitted as PE fillers
between the score/PV matmuls of block j's attention, so the Tensor engine
never drains (stays at max p-state) while the Scalar engine runs exp.
Head pairs share one ot tile (odd head's PV lands on PSUM partitions
63:128 via the [1|V] augmented-V layout) so Wo needs 2 full-K matmuls
per tile instead of 4 half-K ones.
"""

import os
from collections import deque
from contextlib import ExitStack

import numpy as np
import ml_dtypes

import concourse.bass as bass
import concourse.mybir as mybir
import concourse.tile as tile

B, S, D, H = 2, 2048, 1024, 16
DK = D // H  # 64
HG = 4  # heads per core
NCOLS = HG * DK  # 256 columns of the projection per core
THETA = 10000.0
SCALE = 1.0 / float(np.sqrt(DK))
N_CORES = 8

F32 = mybir.dt.float32
F32R = mybir.dt.float32r
BF16 = mybir.dt.bfloat16

SB = 512            # sq block width
NSB = S // SB       # 4
NST = S // 128      # 16 key tiles / V tiles
NDC = D // 128      # 8 contraction chunks
GW = 1              # key tiles per score-psum group
AUGW = DK + 8       # V head stride padded so each head's lhsT is 16B aligned

BF = ml_dtypes.bfloat16


def to_bf16(a):
    return np.ascontiguousarray(np.asarray(a, dtype=np.float32)).astype(BF)


# ---------------------------------------------------------------------------
# host-side prep
# ---------------------------------------------------------------------------

def _rope_perm():
    """Within-head column permutation pi: new row r -> original dk index."""
    perm = np.empty(DK, dtype=np.int64)
    for r in range(DK):
        q, m = divmod(r, 32)
        if m < 16:
            perm[r] = 2 * (16 * q + m)
        else:
            perm[r] = 2 * (16 * q + m - 16) + 1
    return perm


_PERM = _rope_perm()
SHUF_MASK = list(range(16, 32)) + list(range(16))  # swap 16-halves per quadrant


def _causal_masks():
    """mask[p, d, f] = (f >= 128*d + p): causal keep-mask for a key tile at
    diagonal offset d within the sq block."""
    p_ = np.arange(128)[:, None, None]
    d_ = np.arange(4)[None, :, None]
    f_ = np.arange(SB)[None, None, :]
    keep = (f_ >= 128 * d_ + p_)
    return to_bf16(keep.astype(np.float32))


_CAUSAL_MASKS = _causal_masks()


def _rope_tables(pos):
    """cosT/sinT [128, S] fp32 for the permuted layout. pos: [S] int."""
    inv_freq = (np.float32(THETA) ** (-(np.arange(0, DK, 2, dtype=np.float32) / np.float32(DK))))  # [32]
    ang = pos.astype(np.float32)[:, None] * inv_freq[None, :]  # [S, 32]
    cos = np.cos(ang)  # [S, 32]
    sin = np.sin(ang)
    cosT = np.empty((128, S), dtype=np.float32)
    sinT = np.empty((128, S), dtype=np.float32)
    for p in range(128):
        r = p % DK
        q, m = divmod(r, 32)
        if m < 16:
            i = 16 * q + m
            sgn = -1.0
        else:
            i = 16 * q + m - 16
            sgn = 1.0
        cosT[p] = cos[:, i]
        sinT[p] = np.float32(sgn) * sin[:, i]
    return cosT, sinT


def make_core_inputs(x, token_position, Wq, bq, Wk, bk, Wv, bv, Wo, bo):
    """Build the 8 per-core input maps."""
    x = np.asarray(x, dtype=np.float32)
    token_position = np.asarray(token_position)
    Wq, Wk, Wv, Wo = (np.asarray(w, dtype=np.float32) for w in (Wq, Wk, Wv, Wo))
    bq, bk, bv = (np.asarray(b_, dtype=np.float32) for b_ in (bq, bk, bv))

    in_maps = []
    tables = {}
    for c in range(N_CORES):
        b, hg = divmod(c, HG)
        heads = range(HG * hg, HG * hg + HG)
        # permuted q/k column indices for this core's heads
        cols_qk = np.concatenate([DK * h + _PERM for h in heads])
        cols_v = np.arange(NCOLS * hg, NCOLS * hg + NCOLS)
        if b not in tables:
            tables[b] = _rope_tables(np.asarray(token_position[b]))
        cosT, sinT = tables[b]
        wo_rows = Wo[cols_v, :]  # [256, 1024]
        in_maps.append({
            "xT": to_bf16(x[b].T),                              # [1024, 2048]
            "wq": to_bf16(Wq[:, cols_qk]),                      # [1024, 256]
            "wk": to_bf16(Wk[:, cols_qk]),
            "wv": to_bf16(Wv[:, cols_v]),
            "wo": to_bf16(wo_rows.reshape(HG, DK, D).transpose(1, 0, 2)),  # [64, 4, 1024]
            "bq": to_bf16(bq[cols_qk][None, :]),                # [1, 256]
            "bk": to_bf16(bk[cols_qk][None, :]),
            "bv": to_bf16(bv[cols_v][None, :]),
            "ones_row": to_bf16(np.ones((1, SB), np.float32)),
            "onesc": to_bf16(np.ones((128, DK), np.float32)),
            "onesr": np.ones((128, DK), np.float32),
            "maskd": _CAUSAL_MASKS,
            "cosT": cosT,
            "sinT": sinT,
        })
    return in_maps


# ---------------------------------------------------------------------------
# device program
# ---------------------------------------------------------------------------

def build_program(with_bias=False):
    from concourse import bacc, library_config
    nc = bacc.Bacc("TRN2", debug=False)

    xT = nc.declare_dram_parameter("xT", [D, S], BF16, isOutput=False).ap()
    wq = nc.declare_dram_parameter("wq", [D, NCOLS], BF16, isOutput=False).ap()
    wk = nc.declare_dram_parameter("wk", [D, NCOLS], BF16, isOutput=False).ap()
    wv = nc.declare_dram_parameter("wv", [D, NCOLS], BF16, isOutput=False).ap()
    wo = nc.declare_dram_parameter("wo", [DK, HG, D], BF16, isOutput=False).ap()
    bq = nc.declare_dram_parameter("bq", [1, NCOLS], BF16, isOutput=False).ap()
    bk = nc.declare_dram_parameter("bk", [1, NCOLS], BF16, isOutput=False).ap()
    bv = nc.declare_dram_parameter("bv", [1, NCOLS], BF16, isOutput=False).ap()
    ones_row_d = nc.declare_dram_parameter("ones_row", [1, SB], BF16, isOutput=False).ap()
    onesc_d = nc.declare_dram_parameter("onesc", [128, DK], BF16, isOutput=False).ap()
    onesr_d = nc.declare_dram_parameter("onesr", [128, DK], F32R, isOutput=False).ap()
    maskd_d = nc.declare_dram_parameter("maskd", [128, 4, SB], BF16, isOutput=False).ap()
    cosT = nc.declare_dram_parameter("cosT", [128, S], F32, isOutput=False).ap()
    sinT = nc.declare_dram_parameter("sinT", [128, S], F32, isOutput=False).ap()
    out = nc.declare_dram_parameter("out", [S, D], F32, isOutput=True).ap()
    debug_dump = os.environ.get("KERNEL_DEBUG_DUMP", "0") == "1"
    if debug_dump:
        dbg_qt = nc.declare_dram_parameter("dbg_qt", [128, SB], F32, isOutput=True).ap()
        dbg_kh = nc.declare_dram_parameter("dbg_kh", [128, SB], F32, isOutput=True).ap()
        dbg_va = nc.declare_dram_parameter("dbg_va", [128, HG * (DK + 8)], F32, isOutput=True).ap()
        dbg_ot = nc.declare_dram_parameter("dbg_ot", [NSB, HG, DK, SB], F32, isOutput=True).ap()
        dbg_den = nc.declare_dram_parameter("dbg_den", [NSB, HG, 2, SB], F32, isOutput=True).ap()

    with tile.TileContext(nc) as tc, ExitStack() as ctx:
        nc.gpsimd.load_library(library_config.proxy)
        const = ctx.enter_context(tc.tile_pool(name="const", bufs=1))
        sbig = ctx.enter_context(tc.tile_pool(name="sbig", bufs=1))
        rtmp = ctx.enter_context(tc.tile_pool(name="rtmp", bufs=2))
        epool = ctx.enter_context(tc.tile_pool(name="epool", bufs=4))
        npool = ctx.enter_context(tc.tile_pool(name="npool", bufs=2))
        opool = ctx.enter_context(tc.tile_pool(name="opool", bufs=2))
        sc_ps = ctx.enter_context(tc.tile_pool(name="sc_ps", bufs=2, space="PSUM"))
        pv_ps = ctx.enter_context(tc.tile_pool(name="pv_ps", bufs=4, space="PSUM"))
        mm_ps = ctx.enter_context(tc.tile_pool(name="mm_ps", bufs=2, space="PSUM"))

        # --- static SBUF tiles
        wq_sb = [const.tile([128, NCOLS], BF16, tag=f"wq{dc}", name=f"wq{dc}")
                 for dc in range(NDC)]
        wk_sb = [const.tile([128, NCOLS], BF16, tag=f"wk{dc}", name=f"wk{dc}")
                 for dc in range(NDC)]
        wv_sb = [const.tile([128, NCOLS], BF16, tag=f"wv{dc}", name=f"wv{dc}")
                 for dc in range(NDC)]
        cos_sb = const.tile([128, S], F32, tag="cos")
        sin_sb = const.tile([128, S], F32, tag="sin")
        wo_sb = const.tile([DK, HG, D], BF16, tag="wo")
        onesc_sb = const.tile([128, DK], BF16, tag="onesc")
        onesr_sb = const.tile([128, DK], F32R, tag="onesr")
        mask_sb = const.tile([128, 4, SB], BF16, tag="maskd")
        if with_bias:
            bq_sb = const.tile([1, NCOLS], BF16, tag="bq")
            bk_sb = const.tile([1, NCOLS], BF16, tag="bk")
            bv_sb = const.tile([1, NCOLS], BF16, tag="bv")
            ones_row = const.tile([1, SB], BF16, tag="ones_row")
        xt = [[sbig.tile([128, SB], BF16, tag=f"xt{sb}_{dc}", name=f"xt{sb}_{dc}")
               for dc in range(NDC)] for sb in range(NSB)]
        # Q^T / K^T per (chunk, sq-block): chunk c holds heads {2c, 2c+1}
        # stacked on partitions (head 2c rows 0:64, head 2c+1 rows 64:128)
        qt = [[sbig.tile([128, SB], BF16, tag=f"qt{c}_{sb}", name=f"qt{c}_{sb}")
               for sb in range(NSB)] for c in range(2)]
        kth = [[sbig.tile([128, SB], BF16, tag=f"kh{c}_{sb}", name=f"kh{c}_{sb}")
                for sb in range(NSB)] for c in range(2)]
        # V augmented per key tile, every head [V(64) | one] so PV row 64
        # accumulates the softmax denominator
        vaug = [sbig.tile([128, HG * AUGW], BF16, tag=f"va{st}", name=f"va{st}")
                for st in range(NST)]
        # normalized O^T per (head, sq-block), rows 0:64
        ot = [[sbig.tile([DK, SB], BF16, tag=f"ot{h}_{j}", name=f"ot{h}_{j}")
               for j in range(NSB)] for h in range(HG)]
        # static staging for the denominator transpose chain; rows 65:96 /
        # unwritten columns are zeroed once and reused by every head
        den_st = sbig.tile([128, SB], F32, tag="den_st")
        rT_st = sbig.tile([128, SB], F32, tag="rT_st")
        nc.vector.memset(den_st[DK:DK + 32, :], 0.0)
        nc.vector.memset(rT_st[DK:DK + 32, :], 0.0)

        # --- DMAs, critical-path first
        for dc in range(NDC):
            nc.sync.dma_start(wq_sb[dc][:], wq[128 * dc:128 * dc + 128, :])
            nc.sync.dma_start(xt[0][dc][:], xT[128 * dc:128 * dc + 128, 0:SB])
            nc.sync.dma_start(wk_sb[dc][:], wk[128 * dc:128 * dc + 128, :])
            if dc == 1:
                # cos/sin feed the first RoPE; masks feed attention(0) tile 0
                nc.sync.dma_start(cos_sb[:], cosT)
                nc.sync.dma_start(sin_sb[:], sinT)
                nc.sync.dma_start(mask_sb[:], maskd_d)
                nc.sync.dma_start(onesc_sb[:], onesc_d)
        for dc in range(NDC):
            nc.sync.dma_start(wv_sb[dc][:], wv[128 * dc:128 * dc + 128, :])
        nc.sync.dma_start(onesr_sb[:], onesr_d)
        for dc in range(NDC):
            nc.sync.dma_start(xt[1][dc][:], xT[128 * dc:128 * dc + 128, SB:2 * SB])
        nc.sync.dma_start(wo_sb[:], wo)
        if with_bias:
            nc.sync.dma_start(bq_sb[:], bq)
            nc.sync.dma_start(bk_sb[:], bk)
            nc.sync.dma_start(bv_sb[:], bv)
            nc.sync.dma_start(ones_row[:], ones_row_d)
        for sb in (2, 3):
            for dc in range(NDC):
                nc.sync.dma_start(xt[sb][dc][:],
                                  xT[128 * dc:128 * dc + 128, SB * sb:SB * sb + SB])

        # --- emission helpers -------------------------------------------
        def emit_qk_chunk(sb, c, wname):
            """Projection chunk c of Q or K for sq block sb, incl. RoPE."""
            w_sb = wq_sb if wname == "q" else wk_sb
            ss = slice(SB * sb, SB * sb + SB)
            ncol = slice(128 * c, 128 * c + 128)
            ps = mm_ps.tile([128, SB], F32, tag="mm", name="ps_qk")
            for dc in range(NDC):
                nc.tensor.matmul(ps[:], w_sb[dc][:, ncol], xt[sb][dc][:],
                                 start=(dc == 0),
                                 stop=(dc == NDC - 1 and not with_bias))
            if with_bias:
                b_sb = bq_sb if wname == "q" else bk_sb
                nc.tensor.matmul(ps[:], b_sb[0:1, ncol], ones_row[0:1, :],
                                 start=False, stop=True)
            # rope: dst = ps*cos + shuffle(ps)*sin
            t_cos = rtmp.tile([128, SB], F32, tag="rc", name="t_cos")
            nc.vector.tensor_mul(t_cos[:], ps[:], cos_sb[:, ss])
            t_shuf = rtmp.tile([128, SB], F32, tag="rs", name="t_shuf")
            nc.vector.stream_shuffle(t_shuf[:], ps[:], SHUF_MASK)
            t_sin = rtmp.tile([128, SB], F32, tag="rm", name="t_sin")
            nc.gpsimd.tensor_mul(t_sin[:], t_shuf[:], sin_sb[:, ss])
            dst = qt[c][sb] if wname == "q" else kth[c][sb]
            nc.vector.tensor_add(dst[:], t_cos[:], t_sin[:])

        def emit_v_st(sb, st4):
            """V projection for one 128-seq tile, scattered into vaug."""
            st = 4 * sb + st4
            ps = mm_ps.tile([128, SB], F32, tag="mm", name="ps_v")
            for dc in range(NDC):
                nc.tensor.matmul(ps[:, 0:NCOLS],
                                 xt[sb][dc][:, 128 * st4:128 * st4 + 128],
                                 wv_sb[dc][:],
                                 start=(dc == 0),
                                 stop=(dc == NDC - 1 and not with_bias))
            if with_bias:
                nc.tensor.matmul(ps[:, 0:NCOLS], ones_row[0:1, 0:128],
                                 bv_sb[0:1, :], start=False, stop=True)
            va = vaug[st][:].rearrange("p (h e) -> p h e", h=HG)
            psv = ps[:, 0:NCOLS].rearrange("p (h k) -> p h k", h=HG)
            nc.vector.tensor_copy(va[:, :, 0:DK], psv[:, :, :])
            nc.vector.tensor_copy(va[:, :, DK], onesc_sb[:, 0:HG])

        wo_copy_tick = [0]

        def emit_wo(st, dc):
            """Output projection for one (128-seq, 512-dmodel) tile."""
            jb = st // 4
            rq = slice(128 * (st % 4), 128 * (st % 4) + 128)
            cols = slice(SB * dc, SB * dc + SB)
            ps = mm_ps.tile([128, SB], F32, tag="mm", name="ps_wo")
            for h in range(HG):
                nc.tensor.matmul(ps[:], ot[h][jb][:, rq], wo_sb[:, h, cols],
                                 start=(h == 0), stop=(h == HG - 1))
            o_sb = opool.tile([128, SB], F32, tag="osb", name="o_sb")
            if wo_copy_tick[0] % 2 == 0:
                nc.vector.tensor_copy(o_sb[:], ps[:])
            else:
                nc.scalar.copy(o_sb[:], ps[:])
            wo_copy_tick[0] += 1
            nc.sync.dma_start(out[128 * st:128 * st + 128, cols], o_sb[:])

        def emit_norm(pv_t, h, j):
            """ot[h][j] = pv V-rows * broadcast(1/denominator).

            The DVE reciprocal is element-serial per lane, so running it on
            the [1, 512] denominator row costs 3.3us. Instead transpose the
            row through 32x32 stream-transpose blocks so the 512 values land
            on 32 partitions (16 per lane), take the reciprocal there
            (~0.2us), and transpose back."""
            nc.vector.tensor_copy(den_st[DK:DK + 1, :], pv_t[DK:DK + 1, :])
            tT = npool.tile([128, SB], F32, tag="tt", name="tT")
            nc.vector.transpose(tT[DK:DK + 32, :], den_st[DK:DK + 32, :])
            rT = npool.tile([128, SB], F32, tag="rt", name="rT")
            nc.vector.memset(rT[DK:DK + 32, :], 0.0)
            nc.vector.reciprocal(rT[DK:DK + 32, 0:SB:32],
                                 tT[DK:DK + 32, 0:SB:32])
            rec = npool.tile([128, SB], F32, tag="rec", name="rec")
            nc.vector.transpose(rec[DK:DK + 32, :], rT[DK:DK + 32, :])
            rec_b = npool.tile([128, SB], BF16, tag="recb", name="rec_b")
            nc.vector.tensor_copy(rec_b[DK:DK + 1, :], rec[DK:DK + 1, :])
            bcp = mm_ps.tile([128, SB], F32, tag="mm", name="bcp")
            nc.tensor.matmul(bcp[0:DK, :], onesc_sb[DK:DK + 1, :],
                             rec_b[DK:DK + 1, :],
                             start=True, stop=True)
            bc = npool.tile([128, SB], F32, tag="bcs", name="bc")
            nc.vector.tensor_copy(bc[0:DK, :], bcp[0:DK, :])
            nc.vector.tensor_mul(ot[h][j][:], pv_t[0:DK, :], bc[0:DK, :])

        # --- projections for block 0 chunk 0 (pair 1's chunks become
        # the first fillers inside pair 0's attention)
        emit_qk_chunk(0, 0, "q")
        emit_qk_chunk(0, 0, "k")
        for st4 in range(4):
            emit_v_st(0, st4)

        # --- main interleaved stream ------------------------------------
        seq = os.environ.get("KERNEL_SEQ", "0") == "1"
        if seq:
            emit_qk_chunk(0, 1, "q")
            emit_qk_chunk(0, 1, "k")
            for nb in range(1, NSB):
                emit_qk_chunk(nb, 0, "q")
                emit_qk_chunk(nb, 0, "k")
                emit_qk_chunk(nb, 1, "q")
                emit_qk_chunk(nb, 1, "k")
                for st4 in range(4):
                    emit_v_st(nb, st4)
        fillers = deque()
        if not seq:
            fillers.append(lambda: emit_qk_chunk(0, 1, "q"))
            fillers.append(lambda: emit_qk_chunk(0, 1, "k"))
        pending_norm = deque()  # closures, flushed after the next sc group
        for j in range(NSB):
            if not seq and j < NSB - 1:
                nb = j + 1
                fillers.append(lambda nb=nb: emit_qk_chunk(nb, 0, "q"))
                fillers.append(lambda nb=nb: emit_qk_chunk(nb, 0, "k"))
                fillers.append(lambda nb=nb: emit_qk_chunk(nb, 1, "q"))
                fillers.append(lambda nb=nb: emit_qk_chunk(nb, 1, "k"))
                for st4 in range(4):
                    fillers.append(lambda nb=nb, st4=st4: emit_v_st(nb, st4))
            if not seq and j > 0:
                for st in range(4 * (j - 1), 4 * j):
                    for dc in range(2):
                        fillers.append(lambda st=st, dc=dc: emit_wo(st, dc))

            # Head-pair interleave: the even and odd head of a chunk run as
            # two independent score/PV streams, doubling the PE's
            # dependency-free lookahead over the Scalar exp latency. PV runs
            # one key tile behind its scores so e(i) is always ready.
            for c in range(2):
                heads = (2 * c, 2 * c + 1)
                pvs = [pv_ps.tile([128, SB], F32, tag="pv", name="pv")
                       for _ in heads]
                ngrp = 4 * (j + 1)

                def emit_sc(i, parity):
                    rows = slice(DK * parity, DK * parity + DK)
                    sc = sc_ps.tile([128, SB], F32, tag="sc", name="sc")
                    nc.tensor.matmul(
                        sc[:],
                        kth[c][i // 4][rows, 128 * (i % 4):128 * (i % 4) + 128],
                        qt[c][j][rows, :],
                        start=True, stop=True)
                    e = epool.tile([128, SB], BF16, tag="e", name="e")
                    nc.scalar.activation(e[:], sc[:],
                                         mybir.ActivationFunctionType.Exp,
                                         scale=SCALE)
                    d = i - 4 * j
                    if d >= 0:  # tile touches the causal diagonal
                        nc.vector.tensor_mul(e[:], e[:], mask_sb[:, d, :])
                    return e

                def emit_pv(i, parity, e):
                    lhs = vaug[i][:].rearrange(
                        "p (h e) -> p h e", h=HG)[:, heads[parity], 0:DK + 1]
                    nc.tensor.matmul(
                        pvs[parity][0:DK + 1, :], lhs, e[:],
                        start=(i == 0), stop=(i == ngrp - 1))

                prev = None
                for g in range(ngrp):
                    cur = (g, emit_sc(g, 0), emit_sc(g, 1))
                    while pending_norm:
                        pending_norm.popleft()()
                    if fillers:
                        fillers.popleft()()
                    if prev is not None:
                        emit_pv(prev[0], 0, prev[1])
                        emit_pv(prev[0], 1, prev[2])
                    prev = cur
                emit_pv(prev[0], 0, prev[1])
                emit_pv(prev[0], 1, prev[2])
                for parity in range(2):
                    pending_norm.append(
                        lambda pv_t=pvs[parity], hh=heads[parity], jj=j:
                            emit_norm(pv_t, hh, jj))

        # --- tail: last normalize + Wo for block 3
        while pending_norm:
            pending_norm.popleft()()
        while fillers:
            fillers.popleft()()
        wo_start = 0 if seq else 4 * (NSB - 1)
        for st in range(wo_start, 4 * NSB):
            for dc in range(2):
                emit_wo(st, dc)

        if debug_dump:
            dq = opool.tile([128, SB], F32, tag="dbg", name="dq")
            nc.vector.tensor_copy(dq[:], qt[0][1][:])
            nc.sync.dma_start(dbg_qt, dq[:])
            dk_ = opool.tile([128, SB], F32, tag="dbg", name="dk_")
            nc.vector.tensor_copy(dk_[:], kth[0][1][:])
            nc.sync.dma_start(dbg_kh, dk_[:])
            dv = opool.tile([128, HG * AUGW], F32, tag="dbgv", name="dv")
            nc.vector.tensor_copy(dv[:], vaug[4][:])
            nc.sync.dma_start(dbg_va, dv[:])
            for jj in range(NSB):
                for hh in range(HG):
                    do = opool.tile([128, SB], F32, tag="dbg", name="do")
                    nc.vector.tensor_copy(do[0:DK, :], ot[hh][jj][:])
                    nc.sync.dma_start(dbg_ot[jj, hh], do[0:DK, :])

    nc.compile()
    return nc


_CACHED_NC = {}


def _get_program(with_bias=False):
    if with_bias not in _CACHED_NC:
        _CACHED_NC[with_bias] = build_program(with_bias=with_bias)
    return _CACHED_NC[with_bias]


# ---------------------------------------------------------------------------
# entry point
# ---------------------------------------------------------------------------

def kernel(x, token_position, Wq, bq, Wk, bk, Wv, bv, Wo, bo, _results=None):
    from concourse.bass_utils import run_bass_kernel_spmd

    in_maps = make_core_inputs(x, token_position, Wq, bq, Wk, bk, Wv, bv, Wo, bo)
    if _results is None:
        with_bias = any(float(np.abs(np.asarray(v)).max()) != 0.0
                        for v in (bq, bk, bv))
        nc = _get_program(with_bias=with_bias)
        res = run_bass_kernel_spmd(nc, in_maps, list(range(N_CORES)))
        _results = [res.results[i]["out"] for i in range(N_CORES)]
    bo = np.asarray(bo, dtype=np.float32)
    out = np.empty((B, S, D), dtype=np.float32)
    for b in range(B):
        acc = _results[HG * b].astype(np.float32)
        for hg in range(1, HG):
            acc = acc + _results[HG * b + hg]
        out[b] = acc + bo[None, :]
    return out


# revision 35
# speedup vs baseline: 1.0224x; 1.0224x over previous
"""Trainium2 Bass kernel: causal multi-head attention with interleaved RoPE.

Problem shapes (hardcoded): x [2, 2048, 1024], 16 heads of dk=64.
Sharding: 8 cores = 2 batches x 4 head-groups (4 heads each). Each core
computes its head-slice Q/K/V projections, RoPE, causal attention, and a
partial output through its Wo row-slice; the host sums the 4 partials per
batch and adds bo.

RoPE trick: attention scores are invariant to any permutation of the dk
axis applied to both Q and K, so the Wq/Wk columns are permuted on the host
into a "quadrant half-split" layout where each rotation pair partner sits
exactly 16 partitions away inside the same 32-partition quadrant. The DVE
stream_shuffle (a per-quadrant 32-way permute) then produces the swapped
operand, and RoPE becomes: rot = q * cosT + shuffle(q) * sinT with
host-precomputed tables (sinT carries the sign).

Schedule: one interleaved instruction stream. Projections for seq-block
j+1 and the Wo output projection for block j-1 are emitted as PE fillers
between the score/PV matmuls of block j's attention, so the Tensor engine
never drains (stays at max p-state) while the Scalar engine runs exp.
Head pairs share one ot tile (odd head's PV lands on PSUM partitions
63:128 via the [1|V] augmented-V layout) so Wo needs 2 full-K matmuls
per tile instead of 4 half-K ones.
"""

import os
from collections import deque
from contextlib import ExitStack

import numpy as np
import ml_dtypes

import concourse.bass as bass
import concourse.mybir as mybir
import concourse.tile as tile

B, S, D, H = 2, 2048, 1024, 16
DK = D // H  # 64
HG = 4  # heads per core
NCOLS = HG * DK  # 256 columns of the projection per core
THETA = 10000.0
SCALE = 1.0 / float(np.sqrt(DK))
N_CORES = 8

F32 = mybir.dt.float32
F32R = mybir.dt.float32r
BF16 = mybir.dt.bfloat16

SB = 512            # sq block width
NSB = S // SB       # 4
NST = S // 128      # 16 key tiles / V tiles
NDC = D // 128      # 8 contraction chunks
GW = 1              # key tiles per score-psum group
AUGW = DK + 8       # V head stride padded so each head's lhsT is 16B aligned

BF = ml_dtypes.bfloat16


def to_bf16(a):
    return np.ascontiguousarray(np.asarray(a, dtype=np.float32)).astype(BF)


# ---------------------------------------------------------------------------
# host-side prep
# ---------------------------------------------------------------------------

def _rope_perm():
    """Within-head column permutation pi: new row r -> original dk index."""
    perm = np.empty(DK, dtype=np.int64)
    for r in range(DK):
        q, m = divmod(r, 32)
        if m < 16:
            perm[r] = 2 * (16 * q + m)
        else:
            perm[r] = 2 * (16 * q + m - 16) + 1
    return perm


_PERM = _rope_perm()
SHUF_MASK = list(range(16, 32)) + list(range(16))  # swap 16-halves per quadrant


def _causal_masks():
    """mask[p, d, f] = (f >= 128*d + p): causal keep-mask for a key tile at
    diagonal offset d within the sq block."""
    p_ = np.arange(128)[:, None, None]
    d_ = np.arange(4)[None, :, None]
    f_ = np.arange(SB)[None, None, :]
    keep = (f_ >= 128 * d_ + p_)
    return to_bf16(keep.astype(np.float32))


_CAUSAL_MASKS = _causal_masks()


def _rope_tables(pos):
    """cosT/sinT [128, S] fp32 for the permuted layout. pos: [S] int."""
    inv_freq = (np.float32(THETA) ** (-(np.arange(0, DK, 2, dtype=np.float32) / np.float32(DK))))  # [32]
    ang = pos.astype(np.float32)[:, None] * inv_freq[None, :]  # [S, 32]
    cos = np.cos(ang)  # [S, 32]
    sin = np.sin(ang)
    cosT = np.empty((128, S), dtype=np.float32)
    sinT = np.empty((128, S), dtype=np.float32)
    for p in range(128):
        r = p % DK
        q, m = divmod(r, 32)
        if m < 16:
            i = 16 * q + m
            sgn = -1.0
        else:
            i = 16 * q + m - 16
            sgn = 1.0
        cosT[p] = cos[:, i]
        sinT[p] = np.float32(sgn) * sin[:, i]
    return cosT, sinT


def make_core_inputs(x, token_position, Wq, bq, Wk, bk, Wv, bv, Wo, bo):
    """Build the 8 per-core input maps."""
    x = np.asarray(x, dtype=np.float32)
    token_position = np.asarray(token_position)
    Wq, Wk, Wv, Wo = (np.asarray(w, dtype=np.float32) for w in (Wq, Wk, Wv, Wo))
    bq, bk, bv = (np.asarray(b_, dtype=np.float32) for b_ in (bq, bk, bv))

    in_maps = []
    tables = {}
    for c in range(N_CORES):
        b, hg = divmod(c, HG)
        heads = range(HG * hg, HG * hg + HG)
        # permuted q/k column indices for this core's heads
        cols_qk = np.concatenate([DK * h + _PERM for h in heads])
        cols_v = np.arange(NCOLS * hg, NCOLS * hg + NCOLS)
        if b not in tables:
            tables[b] = _rope_tables(np.asarray(token_position[b]))
        cosT, sinT = tables[b]
        wo_rows = Wo[cols_v, :]  # [256, 1024]
        in_maps.append({
            "xT": to_bf16(x[b].T),                              # [1024, 2048]
            "wq": to_bf16(Wq[:, cols_qk]),                      # [1024, 256]
            "wk": to_bf16(Wk[:, cols_qk]),
            "wv": to_bf16(Wv[:, cols_v]),
            "wo": to_bf16(wo_rows.reshape(HG, DK, D).transpose(1, 0, 2)),  # [64, 4, 1024]
            "bq": to_bf16(bq[cols_qk][None, :]),                # [1, 256]
            "bk": to_bf16(bk[cols_qk][None, :]),
            "bv": to_bf16(bv[cols_v][None, :]),
            "ones_row": to_bf16(np.ones((1, SB), np.float32)),
            "onesc": to_bf16(np.ones((128, DK), np.float32)),
            "onesr": np.ones((128, DK), np.float32),
            "maskd": _CAUSAL_MASKS,
            "cosT": cosT,
            "sinT": sinT,
        })
    return in_maps


# ---------------------------------------------------------------------------
# device program
# ---------------------------------------------------------------------------

def build_program(with_bias=False):
    from concourse import bacc, library_config
    nc = bacc.Bacc("TRN2", debug=False)

    xT = nc.declare_dram_parameter("xT", [D, S], BF16, isOutput=False).ap()
    wq = nc.declare_dram_parameter("wq", [D, NCOLS], BF16, isOutput=False).ap()
    wk = nc.declare_dram_parameter("wk", [D, NCOLS], BF16, isOutput=False).ap()
    wv = nc.declare_dram_parameter("wv", [D, NCOLS], BF16, isOutput=False).ap()
    wo = nc.declare_dram_parameter("wo", [DK, HG, D], BF16, isOutput=False).ap()
    bq = nc.declare_dram_parameter("bq", [1, NCOLS], BF16, isOutput=False).ap()
    bk = nc.declare_dram_parameter("bk", [1, NCOLS], BF16, isOutput=False).ap()
    bv = nc.declare_dram_parameter("bv", [1, NCOLS], BF16, isOutput=False).ap()
    ones_row_d = nc.declare_dram_parameter("ones_row", [1, SB], BF16, isOutput=False).ap()
    onesc_d = nc.declare_dram_parameter("onesc", [128, DK], BF16, isOutput=False).ap()
    onesr_d = nc.declare_dram_parameter("onesr", [128, DK], F32R, isOutput=False).ap()
    maskd_d = nc.declare_dram_parameter("maskd", [128, 4, SB], BF16, isOutput=False).ap()
    cosT = nc.declare_dram_parameter("cosT", [128, S], F32, isOutput=False).ap()
    sinT = nc.declare_dram_parameter("sinT", [128, S], F32, isOutput=False).ap()
    out = nc.declare_dram_parameter("out", [S, D], F32, isOutput=True).ap()
    debug_dump = os.environ.get("KERNEL_DEBUG_DUMP", "0") == "1"
    if debug_dump:
        dbg_qt = nc.declare_dram_parameter("dbg_qt", [128, SB], F32, isOutput=True).ap()
        dbg_kh = nc.declare_dram_parameter("dbg_kh", [128, SB], F32, isOutput=True).ap()
        dbg_va = nc.declare_dram_parameter("dbg_va", [128, HG * (DK + 8)], F32, isOutput=True).ap()
        dbg_ot = nc.declare_dram_parameter("dbg_ot", [NSB, HG, DK, SB], F32, isOutput=True).ap()
        dbg_den = nc.declare_dram_parameter("dbg_den", [NSB, HG, 2, SB], F32, isOutput=True).ap()

    with tile.TileContext(nc) as tc, ExitStack() as ctx:
        nc.gpsimd.load_library(library_config.proxy)
        const = ctx.enter_context(tc.tile_pool(name="const", bufs=1))
        sbig = ctx.enter_context(tc.tile_pool(name="sbig", bufs=1))
        rtmp = ctx.enter_context(tc.tile_pool(name="rtmp", bufs=2))
        epool = ctx.enter_context(tc.tile_pool(name="epool", bufs=4))
        npool = ctx.enter_context(tc.tile_pool(name="npool", bufs=2))
        opool = ctx.enter_context(tc.tile_pool(name="opool", bufs=2))
        sc_ps = ctx.enter_context(tc.tile_pool(name="sc_ps", bufs=2, space="PSUM"))
        pv_ps = ctx.enter_context(tc.tile_pool(name="pv_ps", bufs=4, space="PSUM"))
        mm_ps = ctx.enter_context(tc.tile_pool(name="mm_ps", bufs=2, space="PSUM"))

        # --- static SBUF tiles
        wq_sb = [const.tile([128, NCOLS], BF16, tag=f"wq{dc}", name=f"wq{dc}")
                 for dc in range(NDC)]
        wk_sb = [const.tile([128, NCOLS], BF16, tag=f"wk{dc}", name=f"wk{dc}")
                 for dc in range(NDC)]
        wv_sb = [const.tile([128, NCOLS], BF16, tag=f"wv{dc}", name=f"wv{dc}")
                 for dc in range(NDC)]
        cos_sb = const.tile([128, S], F32, tag="cos")
        sin_sb = const.tile([128, S], F32, tag="sin")
        wo_sb = const.tile([DK, HG, D], BF16, tag="wo")
        onesc_sb = const.tile([128, DK], BF16, tag="onesc")
        onesr_sb = const.tile([128, DK], F32R, tag="onesr")
        mask_sb = const.tile([128, 4, SB], BF16, tag="maskd")
        if with_bias:
            bq_sb = const.tile([1, NCOLS], BF16, tag="bq")
            bk_sb = const.tile([1, NCOLS], BF16, tag="bk")
            bv_sb = const.tile([1, NCOLS], BF16, tag="bv")
            ones_row = const.tile([1, SB], BF16, tag="ones_row")
        xt = [[sbig.tile([128, SB], BF16, tag=f"xt{sb}_{dc}", name=f"xt{sb}_{dc}")
               for dc in range(NDC)] for sb in range(NSB)]
        # Q^T / K^T per (chunk, sq-block): chunk c holds heads {2c, 2c+1}
        # stacked on partitions (head 2c rows 0:64, head 2c+1 rows 64:128)
        qt = [[sbig.tile([128, SB], BF16, tag=f"qt{c}_{sb}", name=f"qt{c}_{sb}")
               for sb in range(NSB)] for c in range(2)]
        kth = [[sbig.tile([128, SB], BF16, tag=f"kh{c}_{sb}", name=f"kh{c}_{sb}")
                for sb in range(NSB)] for c in range(2)]
        # V augmented per key tile, every head [V(64) | one] so PV row 64
        # accumulates the softmax denominator
        vaug = [sbig.tile([128, HG * AUGW], BF16, tag=f"va{st}", name=f"va{st}")
                for st in range(NST)]
        # normalized O^T per (head, sq-block), rows 0:64
        ot = [[sbig.tile([DK, SB], BF16, tag=f"ot{h}_{j}", name=f"ot{h}_{j}")
               for j in range(NSB)] for h in range(HG)]


        # --- DMAs, critical-path first
        for dc in range(NDC):
            nc.sync.dma_start(wq_sb[dc][:], wq[128 * dc:128 * dc + 128, :])
            nc.sync.dma_start(xt[0][dc][:], xT[128 * dc:128 * dc + 128, 0:SB])
            nc.sync.dma_start(wk_sb[dc][:], wk[128 * dc:128 * dc + 128, :])
            if dc == 1:
                # cos/sin feed the first RoPE; masks feed attention(0) tile 0
                nc.sync.dma_start(cos_sb[:], cosT)
                nc.sync.dma_start(sin_sb[:], sinT)
                nc.sync.dma_start(mask_sb[:], maskd_d)
                nc.sync.dma_start(onesc_sb[:], onesc_d)
        for dc in range(NDC):
            nc.sync.dma_start(wv_sb[dc][:], wv[128 * dc:128 * dc + 128, :])
        nc.sync.dma_start(onesr_sb[:], onesr_d)
        for dc in range(NDC):
            nc.sync.dma_start(xt[1][dc][:], xT[128 * dc:128 * dc + 128, SB:2 * SB])
        nc.sync.dma_start(wo_sb[:], wo)
        if with_bias:
            nc.sync.dma_start(bq_sb[:], bq)
            nc.sync.dma_start(bk_sb[:], bk)
            nc.sync.dma_start(bv_sb[:], bv)
            nc.sync.dma_start(ones_row[:], ones_row_d)
        for sb in (2, 3):
            for dc in range(NDC):
                nc.sync.dma_start(xt[sb][dc][:],
                                  xT[128 * dc:128 * dc + 128, SB * sb:SB * sb + SB])

        # --- emission helpers -------------------------------------------
        def emit_qk_chunk(sb, c, wname):
            """Projection chunk c of Q or K for sq block sb, incl. RoPE."""
            w_sb = wq_sb if wname == "q" else wk_sb
            ss = slice(SB * sb, SB * sb + SB)
            ncol = slice(128 * c, 128 * c + 128)
            ps = mm_ps.tile([128, SB], F32, tag="mm", name="ps_qk")
            for dc in range(NDC):
                nc.tensor.matmul(ps[:], w_sb[dc][:, ncol], xt[sb][dc][:],
                                 start=(dc == 0),
                                 stop=(dc == NDC - 1 and not with_bias))
            if with_bias:
                b_sb = bq_sb if wname == "q" else bk_sb
                nc.tensor.matmul(ps[:], b_sb[0:1, ncol], ones_row[0:1, :],
                                 start=False, stop=True)
            # rope: dst = ps*cos + shuffle(ps)*sin
            t_cos = rtmp.tile([128, SB], F32, tag="rc", name="t_cos")
            nc.vector.tensor_mul(t_cos[:], ps[:], cos_sb[:, ss])
            t_shuf = rtmp.tile([128, SB], F32, tag="rs", name="t_shuf")
            nc.vector.stream_shuffle(t_shuf[:], ps[:], SHUF_MASK)
            t_sin = rtmp.tile([128, SB], F32, tag="rm", name="t_sin")
            nc.gpsimd.tensor_mul(t_sin[:], t_shuf[:], sin_sb[:, ss])
            dst = qt[c][sb] if wname == "q" else kth[c][sb]
            nc.vector.tensor_add(dst[:], t_cos[:], t_sin[:])

        def emit_v_st(sb, st4):
            """V projection for one 128-seq tile, scattered into vaug."""
            st = 4 * sb + st4
            ps = mm_ps.tile([128, SB], F32, tag="mm", name="ps_v")
            for dc in range(NDC):
                nc.tensor.matmul(ps[:, 0:NCOLS],
                                 xt[sb][dc][:, 128 * st4:128 * st4 + 128],
                                 wv_sb[dc][:],
                                 start=(dc == 0),
                                 stop=(dc == NDC - 1 and not with_bias))
            if with_bias:
                nc.tensor.matmul(ps[:, 0:NCOLS], ones_row[0:1, 0:128],
                                 bv_sb[0:1, :], start=False, stop=True)
            va = vaug[st][:].rearrange("p (h e) -> p h e", h=HG)
            psv = ps[:, 0:NCOLS].rearrange("p (h k) -> p h k", h=HG)
            nc.vector.tensor_copy(va[:, :, 0:DK], psv[:, :, :])
            nc.vector.tensor_copy(va[:, :, DK], onesc_sb[:, 0:HG])

        wo_copy_tick = [0]

        def emit_wo(st, dc):
            """Output projection for one (128-seq, 512-dmodel) tile."""
            jb = st // 4
            rq = slice(128 * (st % 4), 128 * (st % 4) + 128)
            cols = slice(SB * dc, SB * dc + SB)
            ps = mm_ps.tile([128, SB], F32, tag="mm", name="ps_wo")
            for h in range(HG):
                nc.tensor.matmul(ps[:], ot[h][jb][:, rq], wo_sb[:, h, cols],
                                 start=(h == 0), stop=(h == HG - 1))
            o_sb = opool.tile([128, SB], F32, tag="osb", name="o_sb")
            if wo_copy_tick[0] % 2 == 0:
                nc.vector.tensor_copy(o_sb[:], ps[:])
            else:
                nc.scalar.copy(o_sb[:], ps[:])
            wo_copy_tick[0] += 1
            nc.sync.dma_start(out[128 * st:128 * st + 128, cols], o_sb[:])

        def emit_norm(pv_t, h, j):
            """ot[h][j] = pv V-rows * broadcast(1/denominator).

            The DVE reciprocal is element-serial per lane, so running it on
            the [1, 512] denominator row costs 3.3us. Instead transpose the
            row through 32x32 stream-transpose blocks so the 512 values land
            on 32 partitions (16 per lane), take the reciprocal there
            (~0.2us), and transpose back."""
            tT = npool.tile([128, SB], F32, tag="tt", name="tT")
            nc.vector.transpose(tT[DK:DK + 32, :], pv_t[DK:DK + 32, :])
            rT = npool.tile([128, SB], F32, tag="rt", name="rT")
            nc.vector.memset(rT[DK:DK + 32, :], 0.0)
            nc.vector.reciprocal(rT[DK:DK + 32, 0:SB:32],
                                 tT[DK:DK + 32, 0:SB:32])
            rec = npool.tile([128, SB], F32, tag="rec", name="rec")
            nc.vector.transpose(rec[DK:DK + 32, :], rT[DK:DK + 32, :])
            rec_b = npool.tile([128, SB], BF16, tag="recb", name="rec_b")
            nc.vector.tensor_copy(rec_b[DK:DK + 1, :], rec[DK:DK + 1, :])
            bcp = mm_ps.tile([128, SB], F32, tag="mm", name="bcp")
            nc.tensor.matmul(bcp[0:DK, :], onesc_sb[DK:DK + 1, :],
                             rec_b[DK:DK + 1, :],
                             start=True, stop=True)
            bc = npool.tile([128, SB], F32, tag="bcs", name="bc")
            nc.vector.tensor_copy(bc[0:DK, :], bcp[0:DK, :])
            nc.vector.tensor_mul(ot[h][j][:], pv_t[0:DK, :], bc[0:DK, :])

        # --- projections for block 0 chunk 0 (pair 1's chunks become
        # the first fillers inside pair 0's attention)
        emit_qk_chunk(0, 0, "q")
        emit_qk_chunk(0, 0, "k")
        for st4 in range(4):
            emit_v_st(0, st4)

        # --- main interleaved stream ------------------------------------
        seq = os.environ.get("KERNEL_SEQ", "0") == "1"
        if seq:
            emit_qk_chunk(0, 1, "q")
            emit_qk_chunk(0, 1, "k")
            for nb in range(1, NSB):
                emit_qk_chunk(nb, 0, "q")
                emit_qk_chunk(nb, 0, "k")
                emit_qk_chunk(nb, 1, "q")
                emit_qk_chunk(nb, 1, "k")
                for st4 in range(4):
                    emit_v_st(nb, st4)
        fillers = deque()
        if not seq:
            fillers.append(lambda: emit_qk_chunk(0, 1, "q"))
            fillers.append(lambda: emit_qk_chunk(0, 1, "k"))
        pending_norm = deque()  # closures, flushed after the next sc group
        for j in range(NSB):
            if not seq and j < NSB - 1:
                nb = j + 1
                fillers.append(lambda nb=nb: emit_qk_chunk(nb, 0, "q"))
                fillers.append(lambda nb=nb: emit_qk_chunk(nb, 0, "k"))
                fillers.append(lambda nb=nb: emit_qk_chunk(nb, 1, "q"))
                fillers.append(lambda nb=nb: emit_qk_chunk(nb, 1, "k"))
                for st4 in range(4):
                    fillers.append(lambda nb=nb, st4=st4: emit_v_st(nb, st4))
            if not seq and j > 0:
                for st in range(4 * (j - 1), 4 * j):
                    for dc in range(2):
                        fillers.append(lambda st=st, dc=dc: emit_wo(st, dc))

            # Head-pair interleave: the even and odd head of a chunk run as
            # two independent score/PV streams, doubling the PE's
            # dependency-free lookahead over the Scalar exp latency. PV runs
            # one key tile behind its scores so e(i) is always ready.
            for c in range(2):
                heads = (2 * c, 2 * c + 1)
                pvs = [pv_ps.tile([128, SB], F32, tag="pv", name="pv")
                       for _ in heads]
                for pv_t in pvs:
                    # rows 65:96 are read (as don't-care lanes) by the
                    # 32x32 transpose in emit_norm; initialize them once.
                    # Row 64 is re-zeroed by the first PV matmul's start=True.
                    nc.vector.memset(pv_t[DK:DK + 32, :], 0.0)
                ngrp = 4 * (j + 1)

                def emit_sc(i, parity):
                    rows = slice(DK * parity, DK * parity + DK)
                    sc = sc_ps.tile([128, SB], F32, tag="sc", name="sc")
                    nc.tensor.matmul(
                        sc[:],
                        kth[c][i // 4][rows, 128 * (i % 4):128 * (i % 4) + 128],
                        qt[c][j][rows, :],
                        start=True, stop=True)
                    e = epool.tile([128, SB], BF16, tag="e", name="e")
                    nc.scalar.activation(e[:], sc[:],
                                         mybir.ActivationFunctionType.Exp,
                                         scale=SCALE)
                    d = i - 4 * j
                    if d >= 0:  # tile touches the causal diagonal
                        nc.vector.tensor_mul(e[:], e[:], mask_sb[:, d, :])
                    return e

                def emit_pv(i, parity, e):
                    lhs = vaug[i][:].rearrange(
                        "p (h e) -> p h e", h=HG)[:, heads[parity], 0:DK + 1]
                    nc.tensor.matmul(
                        pvs[parity][0:DK + 1, :], lhs, e[:],
                        start=(i == 0), stop=(i == ngrp - 1))

                prev = None
                for g in range(ngrp):
                    cur = (g, emit_sc(g, 0), emit_sc(g, 1))
                    while pending_norm:
                        pending_norm.popleft()()
                    if fillers:
                        fillers.popleft()()
                    if prev is not None:
                        emit_pv(prev[0], 0, prev[1])
                        emit_pv(prev[0], 1, prev[2])
                    prev = cur
                emit_pv(prev[0], 0, prev[1])
                emit_pv(prev[0], 1, prev[2])
                for parity in range(2):
                    pending_norm.append(
                        lambda pv_t=pvs[parity], hh=heads[parity], jj=j:
                            emit_norm(pv_t, hh, jj))

        # --- tail: last normalize + Wo for block 3
        while pending_norm:
            pending_norm.popleft()()
        while fillers:
            fillers.popleft()()
        wo_start = 0 if seq else 4 * (NSB - 1)
        for st in range(wo_start, 4 * NSB):
            for dc in range(2):
                emit_wo(st, dc)

        if debug_dump:
            dq = opool.tile([128, SB], F32, tag="dbg", name="dq")
            nc.vector.tensor_copy(dq[:], qt[0][1][:])
            nc.sync.dma_start(dbg_qt, dq[:])
            dk_ = opool.tile([128, SB], F32, tag="dbg", name="dk_")
            nc.vector.tensor_copy(dk_[:], kth[0][1][:])
            nc.sync.dma_start(dbg_kh, dk_[:])
            dv = opool.tile([128, HG * AUGW], F32, tag="dbgv", name="dv")
            nc.vector.tensor_copy(dv[:], vaug[4][:])
            nc.sync.dma_start(dbg_va, dv[:])
            for jj in range(NSB):
                for hh in range(HG):
                    do = opool.tile([128, SB], F32, tag="dbg", name="do")
                    nc.vector.tensor_copy(do[0:DK, :], ot[hh][jj][:])
                    nc.sync.dma_start(dbg_ot[jj, hh], do[0:DK, :])

    nc.compile()
    return nc


_CACHED_NC = {}


def _get_program(with_bias=False):
    if with_bias not in _CACHED_NC:
        _CACHED_NC[with_bias] = build_program(with_bias=with_bias)
    return _CACHED_NC[with_bias]


# ---------------------------------------------------------------------------
# entry point
# ---------------------------------------------------------------------------

def kernel(x, token_position, Wq, bq, Wk, bk, Wv, bv, Wo, bo, _results=None):
    from concourse.bass_utils import run_bass_kernel_spmd

    in_maps = make_core_inputs(x, token_position, Wq, bq, Wk, bk, Wv, bv, Wo, bo)
    if _results is None:
        with_bias = any(float(np.abs(np.asarray(v)).max()) != 0.0
                        for v in (bq, bk, bv))
        nc = _get_program(with_bias=with_bias)
        res = run_bass_kernel_spmd(nc, in_maps, list(range(N_CORES)))
        _results = [res.results[i]["out"] for i in range(N_CORES)]
    bo = np.asarray(bo, dtype=np.float32)
    out = np.empty((B, S, D), dtype=np.float32)
    for b in range(B):
        acc = _results[HG * b].astype(np.float32)
        for hg in range(1, HG):
            acc = acc + _results[HG * b + hg]
        out[b] = acc + bo[None, :]
    return out


# revision 36
# speedup vs baseline: 1.0812x; 1.0575x over previous
"""Trainium2 Bass kernel: causal multi-head attention with interleaved RoPE.

Problem shapes (hardcoded): x [2, 2048, 1024], 16 heads of dk=64.
Sharding: 8 cores = 2 batches x 4 head-groups (4 heads each). Each core
computes its head-slice Q/K/V projections, RoPE, causal attention, and a
partial output through its Wo row-slice; the host sums the 4 partials per
batch and adds bo.

RoPE trick: attention scores are invariant to any permutation of the dk
axis applied to both Q and K, so the Wq/Wk columns are permuted on the host
into a "quadrant half-split" layout where each rotation pair partner sits
exactly 16 partitions away inside the same 32-partition quadrant. The DVE
stream_shuffle (a per-quadrant 32-way permute) then produces the swapped
operand, and RoPE becomes: rot = q * cosT + shuffle(q) * sinT with
host-precomputed tables (sinT carries the sign).

Schedule: one interleaved instruction stream. Projections for seq-block
j+1 and the Wo output projection for block j-1 are emitted as PE fillers
between the score/PV matmuls of block j's attention, so the Tensor engine
never drains (stays at max p-state) while the Scalar engine runs exp.
Head pairs share one ot tile (odd head's PV lands on PSUM partitions
63:128 via the [1|V] augmented-V layout) so Wo needs 2 full-K matmuls
per tile instead of 4 half-K ones.
"""

import os
from collections import deque
from contextlib import ExitStack

import numpy as np
import ml_dtypes

import concourse.bass as bass
import concourse.mybir as mybir
import concourse.tile as tile

B, S, D, H = 2, 2048, 1024, 16
DK = D // H  # 64
HG = 4  # heads per core
NCOLS = HG * DK  # 256 columns of the projection per core
THETA = 10000.0
SCALE = 1.0 / float(np.sqrt(DK))
N_CORES = 8

F32 = mybir.dt.float32
F32R = mybir.dt.float32r
BF16 = mybir.dt.bfloat16

SB = 512            # sq block width
NSB = S // SB       # 4
NST = S // 128      # 16 key tiles / V tiles
NDC = D // 128      # 8 contraction chunks
GW = 1              # key tiles per score-psum group
AUGW = DK + 8       # V head stride padded so each head's lhsT is 16B aligned

BF = ml_dtypes.bfloat16


def to_bf16(a):
    return np.ascontiguousarray(np.asarray(a, dtype=np.float32)).astype(BF)


# ---------------------------------------------------------------------------
# host-side prep
# ---------------------------------------------------------------------------

def _rope_perm():
    """Within-head column permutation pi: new row r -> original dk index."""
    perm = np.empty(DK, dtype=np.int64)
    for r in range(DK):
        q, m = divmod(r, 32)
        if m < 16:
            perm[r] = 2 * (16 * q + m)
        else:
            perm[r] = 2 * (16 * q + m - 16) + 1
    return perm


_PERM = _rope_perm()
SHUF_MASK = list(range(16, 32)) + list(range(16))  # swap 16-halves per quadrant


def _causal_masks():
    """mask[p, d, f] = (f >= 128*d + p): causal keep-mask for a key tile at
    diagonal offset d within the sq block."""
    p_ = np.arange(128)[:, None, None]
    d_ = np.arange(4)[None, :, None]
    f_ = np.arange(SB)[None, None, :]
    keep = (f_ >= 128 * d_ + p_)
    return to_bf16(keep.astype(np.float32))


_CAUSAL_MASKS = _causal_masks()


def _rope_tables(pos):
    """cosT/sinT [128, S] fp32 for the permuted layout. pos: [S] int."""
    inv_freq = (np.float32(THETA) ** (-(np.arange(0, DK, 2, dtype=np.float32) / np.float32(DK))))  # [32]
    ang = pos.astype(np.float32)[:, None] * inv_freq[None, :]  # [S, 32]
    cos = np.cos(ang)  # [S, 32]
    sin = np.sin(ang)
    cosT = np.empty((128, S), dtype=np.float32)
    sinT = np.empty((128, S), dtype=np.float32)
    for p in range(128):
        r = p % DK
        q, m = divmod(r, 32)
        if m < 16:
            i = 16 * q + m
            sgn = -1.0
        else:
            i = 16 * q + m - 16
            sgn = 1.0
        cosT[p] = cos[:, i]
        sinT[p] = np.float32(sgn) * sin[:, i]
    return cosT, sinT


def make_core_inputs(x, token_position, Wq, bq, Wk, bk, Wv, bv, Wo, bo):
    """Build the 8 per-core input maps."""
    x = np.asarray(x, dtype=np.float32)
    token_position = np.asarray(token_position)
    Wq, Wk, Wv, Wo = (np.asarray(w, dtype=np.float32) for w in (Wq, Wk, Wv, Wo))
    bq, bk, bv = (np.asarray(b_, dtype=np.float32) for b_ in (bq, bk, bv))

    in_maps = []
    tables = {}
    for c in range(N_CORES):
        b, hg = divmod(c, HG)
        heads = range(HG * hg, HG * hg + HG)
        # permuted q/k column indices for this core's heads
        cols_qk = np.concatenate([DK * h + _PERM for h in heads])
        cols_v = np.arange(NCOLS * hg, NCOLS * hg + NCOLS)
        if b not in tables:
            tables[b] = _rope_tables(np.asarray(token_position[b]))
        cosT, sinT = tables[b]
        wo_rows = Wo[cols_v, :]  # [256, 1024]
        in_maps.append({
            "xT": to_bf16(x[b].T),                              # [1024, 2048]
            "wq": to_bf16(Wq[:, cols_qk]),                      # [1024, 256]
            "wk": to_bf16(Wk[:, cols_qk]),
            "wv": to_bf16(Wv[:, cols_v]),
            "wo": to_bf16(wo_rows.reshape(HG, DK, D).transpose(1, 0, 2)),  # [64, 4, 1024]
            "bq": to_bf16(bq[cols_qk][None, :]),                # [1, 256]
            "bk": to_bf16(bk[cols_qk][None, :]),
            "bv": to_bf16(bv[cols_v][None, :]),
            "ones_row": to_bf16(np.ones((1, SB), np.float32)),
            "onesc": to_bf16(np.ones((128, DK), np.float32)),
            "onesr": np.ones((128, DK), np.float32),
            "maskd": _CAUSAL_MASKS,
            "cosT": cosT,
            "sinT": sinT,
        })
    return in_maps


# ---------------------------------------------------------------------------
# device program
# ---------------------------------------------------------------------------

def build_program(with_bias=False):
    from concourse import bacc, library_config
    nc = bacc.Bacc("TRN2", debug=False)

    xT = nc.declare_dram_parameter("xT", [D, S], BF16, isOutput=False).ap()
    wq = nc.declare_dram_parameter("wq", [D, NCOLS], BF16, isOutput=False).ap()
    wk = nc.declare_dram_parameter("wk", [D, NCOLS], BF16, isOutput=False).ap()
    wv = nc.declare_dram_parameter("wv", [D, NCOLS], BF16, isOutput=False).ap()
    wo = nc.declare_dram_parameter("wo", [DK, HG, D], BF16, isOutput=False).ap()
    bq = nc.declare_dram_parameter("bq", [1, NCOLS], BF16, isOutput=False).ap()
    bk = nc.declare_dram_parameter("bk", [1, NCOLS], BF16, isOutput=False).ap()
    bv = nc.declare_dram_parameter("bv", [1, NCOLS], BF16, isOutput=False).ap()
    ones_row_d = nc.declare_dram_parameter("ones_row", [1, SB], BF16, isOutput=False).ap()
    onesc_d = nc.declare_dram_parameter("onesc", [128, DK], BF16, isOutput=False).ap()
    onesr_d = nc.declare_dram_parameter("onesr", [128, DK], F32R, isOutput=False).ap()
    maskd_d = nc.declare_dram_parameter("maskd", [128, 4, SB], BF16, isOutput=False).ap()
    cosT = nc.declare_dram_parameter("cosT", [128, S], F32, isOutput=False).ap()
    sinT = nc.declare_dram_parameter("sinT", [128, S], F32, isOutput=False).ap()
    out = nc.declare_dram_parameter("out", [S, D], F32, isOutput=True).ap()
    debug_dump = os.environ.get("KERNEL_DEBUG_DUMP", "0") == "1"
    if debug_dump:
        dbg_qt = nc.declare_dram_parameter("dbg_qt", [128, SB], F32, isOutput=True).ap()
        dbg_kh = nc.declare_dram_parameter("dbg_kh", [128, SB], F32, isOutput=True).ap()
        dbg_va = nc.declare_dram_parameter("dbg_va", [128, HG * (DK + 8)], F32, isOutput=True).ap()
        dbg_ot = nc.declare_dram_parameter("dbg_ot", [NSB, HG, DK, SB], F32, isOutput=True).ap()
        dbg_den = nc.declare_dram_parameter("dbg_den", [NSB, HG, 2, SB], F32, isOutput=True).ap()

    with tile.TileContext(nc) as tc, ExitStack() as ctx:
        nc.gpsimd.load_library(library_config.proxy)
        const = ctx.enter_context(tc.tile_pool(name="const", bufs=1))
        sbig = ctx.enter_context(tc.tile_pool(name="sbig", bufs=1))
        rtmp = ctx.enter_context(tc.tile_pool(name="rtmp", bufs=2))
        epool = ctx.enter_context(tc.tile_pool(name="epool", bufs=4))
        npool = ctx.enter_context(tc.tile_pool(name="npool", bufs=2))
        opool = ctx.enter_context(tc.tile_pool(name="opool", bufs=2))
        sc_ps = ctx.enter_context(tc.tile_pool(name="sc_ps", bufs=2, space="PSUM"))
        pv_ps = ctx.enter_context(tc.tile_pool(name="pv_ps", bufs=4, space="PSUM"))
        mm_ps = ctx.enter_context(tc.tile_pool(name="mm_ps", bufs=2, space="PSUM"))

        # --- static SBUF tiles
        wq_sb = [const.tile([128, NCOLS], BF16, tag=f"wq{dc}", name=f"wq{dc}")
                 for dc in range(NDC)]
        wk_sb = [const.tile([128, NCOLS], BF16, tag=f"wk{dc}", name=f"wk{dc}")
                 for dc in range(NDC)]
        wv_sb = [const.tile([128, NCOLS], BF16, tag=f"wv{dc}", name=f"wv{dc}")
                 for dc in range(NDC)]
        cos_sb = const.tile([128, S], F32, tag="cos")
        sin_sb = const.tile([128, S], F32, tag="sin")
        wo_sb = const.tile([DK, HG, D], BF16, tag="wo")
        onesc_sb = const.tile([128, DK], BF16, tag="onesc")
        onesr_sb = const.tile([128, DK], F32R, tag="onesr")
        mask_sb = const.tile([128, 4, SB], BF16, tag="maskd")
        if with_bias:
            bq_sb = const.tile([1, NCOLS], BF16, tag="bq")
            bk_sb = const.tile([1, NCOLS], BF16, tag="bk")
            bv_sb = const.tile([1, NCOLS], BF16, tag="bv")
            ones_row = const.tile([1, SB], BF16, tag="ones_row")
        xt = [[sbig.tile([128, SB], BF16, tag=f"xt{sb}_{dc}", name=f"xt{sb}_{dc}")
               for dc in range(NDC)] for sb in range(NSB)]
        # Q^T / K^T per (chunk, sq-block): chunk c holds heads {2c, 2c+1}
        # stacked on partitions (head 2c rows 0:64, head 2c+1 rows 64:128)
        qt = [[sbig.tile([128, SB], BF16, tag=f"qt{c}_{sb}", name=f"qt{c}_{sb}")
               for sb in range(NSB)] for c in range(2)]
        kth = [[sbig.tile([128, SB], BF16, tag=f"kh{c}_{sb}", name=f"kh{c}_{sb}")
                for sb in range(NSB)] for c in range(2)]
        # V augmented per key tile, every head [V(64) | one] so PV row 64
        # accumulates the softmax denominator
        vaug = [sbig.tile([128, HG * AUGW], BF16, tag=f"va{st}", name=f"va{st}")
                for st in range(NST)]
        # normalized O^T per (head, sq-block), rows 0:64
        ot = [[sbig.tile([DK, SB], BF16, tag=f"ot{h}_{j}", name=f"ot{h}_{j}")
               for j in range(NSB)] for h in range(HG)]


        # --- DMAs, critical-path first
        for dc in range(NDC):
            nc.sync.dma_start(wq_sb[dc][:], wq[128 * dc:128 * dc + 128, :])
            nc.sync.dma_start(xt[0][dc][:], xT[128 * dc:128 * dc + 128, 0:SB])
        nc.sync.dma_start(cos_sb[:], cosT)
        nc.sync.dma_start(sin_sb[:], sinT)
        for dc in range(NDC):
            nc.sync.dma_start(wk_sb[dc][:], wk[128 * dc:128 * dc + 128, :])
        nc.sync.dma_start(mask_sb[:], maskd_d)
        nc.sync.dma_start(onesc_sb[:], onesc_d)
        for dc in range(NDC):
            nc.sync.dma_start(wv_sb[dc][:], wv[128 * dc:128 * dc + 128, :])
        nc.sync.dma_start(onesr_sb[:], onesr_d)
        for dc in range(NDC):
            nc.sync.dma_start(xt[1][dc][:], xT[128 * dc:128 * dc + 128, SB:2 * SB])
        nc.sync.dma_start(wo_sb[:], wo)
        if with_bias:
            nc.sync.dma_start(bq_sb[:], bq)
            nc.sync.dma_start(bk_sb[:], bk)
            nc.sync.dma_start(bv_sb[:], bv)
            nc.sync.dma_start(ones_row[:], ones_row_d)
        for sb in (2, 3):
            for dc in range(NDC):
                nc.sync.dma_start(xt[sb][dc][:],
                                  xT[128 * dc:128 * dc + 128, SB * sb:SB * sb + SB])

        # --- emission helpers -------------------------------------------
        def emit_qk_chunk(sb, c, wname):
            """Projection chunk c of Q or K for sq block sb, incl. RoPE."""
            w_sb = wq_sb if wname == "q" else wk_sb
            ss = slice(SB * sb, SB * sb + SB)
            ncol = slice(128 * c, 128 * c + 128)
            ps = mm_ps.tile([128, SB], F32, tag="mm", name="ps_qk")
            for dc in range(NDC):
                nc.tensor.matmul(ps[:], w_sb[dc][:, ncol], xt[sb][dc][:],
                                 start=(dc == 0),
                                 stop=(dc == NDC - 1 and not with_bias))
            if with_bias:
                b_sb = bq_sb if wname == "q" else bk_sb
                nc.tensor.matmul(ps[:], b_sb[0:1, ncol], ones_row[0:1, :],
                                 start=False, stop=True)
            # rope: dst = ps*cos + shuffle(ps)*sin
            t_cos = rtmp.tile([128, SB], F32, tag="rc", name="t_cos")
            nc.vector.tensor_mul(t_cos[:], ps[:], cos_sb[:, ss])
            t_shuf = rtmp.tile([128, SB], F32, tag="rs", name="t_shuf")
            nc.vector.stream_shuffle(t_shuf[:], ps[:], SHUF_MASK)
            t_sin = rtmp.tile([128, SB], F32, tag="rm", name="t_sin")
            nc.gpsimd.tensor_mul(t_sin[:], t_shuf[:], sin_sb[:, ss])
            dst = qt[c][sb] if wname == "q" else kth[c][sb]
            nc.vector.tensor_add(dst[:], t_cos[:], t_sin[:])

        def emit_v_st(sb, st4):
            """V projection for one 128-seq tile, scattered into vaug."""
            st = 4 * sb + st4
            ps = mm_ps.tile([128, SB], F32, tag="mm", name="ps_v")
            for dc in range(NDC):
                nc.tensor.matmul(ps[:, 0:NCOLS],
                                 xt[sb][dc][:, 128 * st4:128 * st4 + 128],
                                 wv_sb[dc][:],
                                 start=(dc == 0),
                                 stop=(dc == NDC - 1 and not with_bias))
            if with_bias:
                nc.tensor.matmul(ps[:, 0:NCOLS], ones_row[0:1, 0:128],
                                 bv_sb[0:1, :], start=False, stop=True)
            va = vaug[st][:].rearrange("p (h e) -> p h e", h=HG)
            psv = ps[:, 0:NCOLS].rearrange("p (h k) -> p h k", h=HG)
            nc.vector.tensor_copy(va[:, :, 0:DK], psv[:, :, :])
            nc.vector.tensor_copy(va[:, :, DK], onesc_sb[:, 0:HG])

        wo_copy_tick = [0]

        def emit_wo(st, dc):
            """Output projection for one (128-seq, 512-dmodel) tile."""
            jb = st // 4
            rq = slice(128 * (st % 4), 128 * (st % 4) + 128)
            cols = slice(SB * dc, SB * dc + SB)
            ps = mm_ps.tile([128, SB], F32, tag="mm", name="ps_wo")
            for h in range(HG):
                nc.tensor.matmul(ps[:], ot[h][jb][:, rq], wo_sb[:, h, cols],
                                 start=(h == 0), stop=(h == HG - 1))
            o_sb = opool.tile([128, SB], F32, tag="osb", name="o_sb")
            if wo_copy_tick[0] % 2 == 0:
                nc.vector.tensor_copy(o_sb[:], ps[:])
            else:
                nc.scalar.copy(o_sb[:], ps[:])
            wo_copy_tick[0] += 1
            nc.sync.dma_start(out[128 * st:128 * st + 128, cols], o_sb[:])

        def emit_norm(pv_t, h, j):
            """ot[h][j] = pv V-rows * broadcast(1/denominator).

            The DVE reciprocal is element-serial per lane, so running it on
            the [1, 512] denominator row costs 3.3us. Instead transpose the
            row through 32x32 stream-transpose blocks so the 512 values land
            on 32 partitions (16 per lane), take the reciprocal there
            (~0.2us), and transpose back."""
            tT = npool.tile([128, SB], F32, tag="tt", name="tT")
            nc.vector.transpose(tT[DK:DK + 32, :], pv_t[DK:DK + 32, :])
            rT = npool.tile([128, SB], F32, tag="rt", name="rT")
            nc.vector.memset(rT[DK:DK + 32, :], 0.0)
            nc.vector.reciprocal(rT[DK:DK + 32, 0:SB:32],
                                 tT[DK:DK + 32, 0:SB:32])
            rec = npool.tile([128, SB], F32, tag="rec", name="rec")
            nc.vector.transpose(rec[DK:DK + 32, :], rT[DK:DK + 32, :])
            rec_b = npool.tile([128, SB], BF16, tag="recb", name="rec_b")
            nc.vector.tensor_copy(rec_b[DK:DK + 1, :], rec[DK:DK + 1, :])
            bcp = mm_ps.tile([128, SB], F32, tag="mm", name="bcp")
            nc.tensor.matmul(bcp[0:DK, :], onesc_sb[DK:DK + 1, :],
                             rec_b[DK:DK + 1, :],
                             start=True, stop=True)
            bc = npool.tile([128, SB], F32, tag="bcs", name="bc")
            nc.vector.tensor_copy(bc[0:DK, :], bcp[0:DK, :])
            nc.vector.tensor_mul(ot[h][j][:], pv_t[0:DK, :], bc[0:DK, :])

        # --- projections for block 0 chunk 0 (pair 1's chunks become
        # the first fillers inside pair 0's attention)
        emit_qk_chunk(0, 0, "q")
        emit_qk_chunk(0, 0, "k")
        for st4 in range(4):
            emit_v_st(0, st4)

        # --- main interleaved stream ------------------------------------
        seq = os.environ.get("KERNEL_SEQ", "0") == "1"
        if seq:
            emit_qk_chunk(0, 1, "q")
            emit_qk_chunk(0, 1, "k")
            for nb in range(1, NSB):
                emit_qk_chunk(nb, 0, "q")
                emit_qk_chunk(nb, 0, "k")
                emit_qk_chunk(nb, 1, "q")
                emit_qk_chunk(nb, 1, "k")
                for st4 in range(4):
                    emit_v_st(nb, st4)
        fillers = deque()
        if not seq:
            fillers.append(lambda: emit_qk_chunk(0, 1, "q"))
            fillers.append(lambda: emit_qk_chunk(0, 1, "k"))
        pending_norm = deque()  # closures, flushed after the next sc group
        for j in range(NSB):
            if not seq and j < NSB - 1:
                nb = j + 1
                fillers.append(lambda nb=nb: emit_qk_chunk(nb, 0, "q"))
                fillers.append(lambda nb=nb: emit_qk_chunk(nb, 0, "k"))
                fillers.append(lambda nb=nb: emit_qk_chunk(nb, 1, "q"))
                fillers.append(lambda nb=nb: emit_qk_chunk(nb, 1, "k"))
                for st4 in range(4):
                    fillers.append(lambda nb=nb, st4=st4: emit_v_st(nb, st4))
            if not seq and j > 0:
                for st in range(4 * (j - 1), 4 * j):
                    for dc in range(2):
                        fillers.append(lambda st=st, dc=dc: emit_wo(st, dc))

            # Head-pair interleave: the even and odd head of a chunk run as
            # two independent score/PV streams, doubling the PE's
            # dependency-free lookahead over the Scalar exp latency. PV runs
            # one key tile behind its scores so e(i) is always ready.
            for c in range(2):
                heads = (2 * c, 2 * c + 1)
                pvs = [pv_ps.tile([128, SB], F32, tag="pv", name="pv")
                       for _ in heads]
                for pv_t in pvs:
                    # rows 65:96 are read (as don't-care lanes) by the
                    # 32x32 transpose in emit_norm; initialize them once.
                    # Row 64 is re-zeroed by the first PV matmul's start=True.
                    nc.vector.memset(pv_t[DK:DK + 32, :], 0.0)
                ngrp = 4 * (j + 1)

                def emit_sc(i, parity):
                    rows = slice(DK * parity, DK * parity + DK)
                    sc = sc_ps.tile([128, SB], F32, tag="sc", name="sc")
                    nc.tensor.matmul(
                        sc[:],
                        kth[c][i // 4][rows, 128 * (i % 4):128 * (i % 4) + 128],
                        qt[c][j][rows, :],
                        start=True, stop=True)
                    e = epool.tile([128, SB], BF16, tag="e", name="e")
                    nc.scalar.activation(e[:], sc[:],
                                         mybir.ActivationFunctionType.Exp,
                                         scale=SCALE)
                    d = i - 4 * j
                    if d >= 0:  # tile touches the causal diagonal
                        nc.vector.tensor_mul(e[:], e[:], mask_sb[:, d, :])
                    return e

                def emit_pv(i, parity, e):
                    lhs = vaug[i][:].rearrange(
                        "p (h e) -> p h e", h=HG)[:, heads[parity], 0:DK + 1]
                    nc.tensor.matmul(
                        pvs[parity][0:DK + 1, :], lhs, e[:],
                        start=(i == 0), stop=(i == ngrp - 1))

                prev = None
                for g in range(ngrp):
                    cur = (g, emit_sc(g, 0), emit_sc(g, 1))
                    while pending_norm:
                        pending_norm.popleft()()
                    if fillers:
                        fillers.popleft()()
                    if prev is not None:
                        emit_pv(prev[0], 0, prev[1])
                        emit_pv(prev[0], 1, prev[2])
                    prev = cur
                emit_pv(prev[0], 0, prev[1])
                emit_pv(prev[0], 1, prev[2])
                for parity in range(2):
                    pending_norm.append(
                        lambda pv_t=pvs[parity], hh=heads[parity], jj=j:
                            emit_norm(pv_t, hh, jj))

        # --- tail: last normalize + Wo for block 3
        while pending_norm:
            pending_norm.popleft()()
        while fillers:
            fillers.popleft()()
        wo_start = 0 if seq else 4 * (NSB - 1)
        for st in range(wo_start, 4 * NSB):
            for dc in range(2):
                emit_wo(st, dc)

        if debug_dump:
            dq = opool.tile([128, SB], F32, tag="dbg", name="dq")
            nc.vector.tensor_copy(dq[:], qt[0][1][:])
            nc.sync.dma_start(dbg_qt, dq[:])
            dk_ = opool.tile([128, SB], F32, tag="dbg", name="dk_")
            nc.vector.tensor_copy(dk_[:], kth[0][1][:])
            nc.sync.dma_start(dbg_kh, dk_[:])
            dv = opool.tile([128, HG * AUGW], F32, tag="dbgv", name="dv")
            nc.vector.tensor_copy(dv[:], vaug[4][:])
            nc.sync.dma_start(dbg_va, dv[:])
            for jj in range(NSB):
                for hh in range(HG):
                    do = opool.tile([128, SB], F32, tag="dbg", name="do")
                    nc.vector.tensor_copy(do[0:DK, :], ot[hh][jj][:])
                    nc.sync.dma_start(dbg_ot[jj, hh], do[0:DK, :])

    nc.compile()
    return nc


_CACHED_NC = {}


def _get_program(with_bias=False):
    if with_bias not in _CACHED_NC:
        _CACHED_NC[with_bias] = build_program(with_bias=with_bias)
    return _CACHED_NC[with_bias]


# ---------------------------------------------------------------------------
# entry point
# ---------------------------------------------------------------------------

def kernel(x, token_position, Wq, bq, Wk, bk, Wv, bv, Wo, bo, _results=None):
    from concourse.bass_utils import run_bass_kernel_spmd

    in_maps = make_core_inputs(x, token_position, Wq, bq, Wk, bk, Wv, bv, Wo, bo)
    if _results is None:
        with_bias = any(float(np.abs(np.asarray(v)).max()) != 0.0
                        for v in (bq, bk, bv))
        nc = _get_program(with_bias=with_bias)
        res = run_bass_kernel_spmd(nc, in_maps, list(range(N_CORES)))
        _results = [res.results[i]["out"] for i in range(N_CORES)]
    bo = np.asarray(bo, dtype=np.float32)
    out = np.empty((B, S, D), dtype=np.float32)
    for b in range(B):
        acc = _results[HG * b].astype(np.float32)
        for hg in range(1, HG):
            acc = acc + _results[HG * b + hg]
        out[b] = acc + bo[None, :]
    return out


# revision 38
# speedup vs baseline: 1.1207x; 1.0365x over previous
"""Trainium2 Bass kernel: causal multi-head attention with interleaved RoPE.

Problem shapes (hardcoded): x [2, 2048, 1024], 16 heads of dk=64.
Sharding: 8 cores = 2 batches x 4 head-groups (4 heads each). Each core
computes its head-slice Q/K/V projections, RoPE, causal attention, and a
partial output through its Wo row-slice; the host sums the 4 partials per
batch and adds bo.

RoPE trick: attention scores are invariant to any permutation of the dk
axis applied to both Q and K, so the Wq/Wk columns are permuted on the host
into a "quadrant half-split" layout where each rotation pair partner sits
exactly 16 partitions away inside the same 32-partition quadrant. The DVE
stream_shuffle (a per-quadrant 32-way permute) then produces the swapped
operand, and RoPE becomes: rot = q * cosT + shuffle(q) * sinT with
host-precomputed tables (sinT carries the sign).

Schedule: one interleaved instruction stream. Projections for seq-block
j+1 and the Wo output projection for block j-1 are emitted as PE fillers
between the score/PV matmuls of block j's attention, so the Tensor engine
never drains (stays at max p-state) while the Scalar engine runs exp.
Head pairs share one ot tile (odd head's PV lands on PSUM partitions
63:128 via the [1|V] augmented-V layout) so Wo needs 2 full-K matmuls
per tile instead of 4 half-K ones.
"""

import os
from collections import deque
from contextlib import ExitStack

import numpy as np
import ml_dtypes

import concourse.bass as bass
import concourse.mybir as mybir
import concourse.tile as tile

B, S, D, H = 2, 2048, 1024, 16
DK = D // H  # 64
HG = 4  # heads per core
NCOLS = HG * DK  # 256 columns of the projection per core
THETA = 10000.0
SCALE = 1.0 / float(np.sqrt(DK))
N_CORES = 8

F32 = mybir.dt.float32
F32R = mybir.dt.float32r
BF16 = mybir.dt.bfloat16

SB = 512            # sq block width
NSB = S // SB       # 4
NST = S // 128      # 16 key tiles / V tiles
NDC = D // 128      # 8 contraction chunks
GW = 1              # key tiles per score-psum group
AUGW = DK + 8       # V head stride padded so each head's lhsT is 16B aligned

BF = ml_dtypes.bfloat16


def to_bf16(a):
    return np.ascontiguousarray(np.asarray(a, dtype=np.float32)).astype(BF)


# ---------------------------------------------------------------------------
# host-side prep
# ---------------------------------------------------------------------------

def _rope_perm():
    """Within-head column permutation pi: new row r -> original dk index."""
    perm = np.empty(DK, dtype=np.int64)
    for r in range(DK):
        q, m = divmod(r, 32)
        if m < 16:
            perm[r] = 2 * (16 * q + m)
        else:
            perm[r] = 2 * (16 * q + m - 16) + 1
    return perm


_PERM = _rope_perm()
SHUF_MASK = list(range(16, 32)) + list(range(16))  # swap 16-halves per quadrant


def _causal_masks():
    """mask[p, d, t, f] = (f >= 128*d + p): causal keep-mask for a key tile
    at diagonal offset d, doubled along t so one multiply covers the fused
    even|odd head pair of score columns."""
    p_ = np.arange(128)[:, None, None, None]
    d_ = np.arange(4)[None, :, None, None]
    f_ = np.arange(SB)[None, None, None, :]
    keep = (f_ >= 128 * d_ + p_) | (np.arange(2)[None, None, :, None] < 0)
    keep = np.broadcast_to(keep, (128, 4, 2, SB)).reshape(128, 4, 2 * SB)
    return to_bf16(keep.astype(np.float32))


_CAUSAL_MASKS = _causal_masks()


def _rope_tables(pos):
    """cosT/sinT [128, S] fp32 for the permuted layout. pos: [S] int."""
    inv_freq = (np.float32(THETA) ** (-(np.arange(0, DK, 2, dtype=np.float32) / np.float32(DK))))  # [32]
    ang = pos.astype(np.float32)[:, None] * inv_freq[None, :]  # [S, 32]
    cos = np.cos(ang)  # [S, 32]
    sin = np.sin(ang)
    cosT = np.empty((128, S), dtype=np.float32)
    sinT = np.empty((128, S), dtype=np.float32)
    for p in range(128):
        r = p % DK
        q, m = divmod(r, 32)
        if m < 16:
            i = 16 * q + m
            sgn = -1.0
        else:
            i = 16 * q + m - 16
            sgn = 1.0
        cosT[p] = cos[:, i]
        sinT[p] = np.float32(sgn) * sin[:, i]
    return cosT, sinT


def make_core_inputs(x, token_position, Wq, bq, Wk, bk, Wv, bv, Wo, bo):
    """Build the 8 per-core input maps."""
    x = np.asarray(x, dtype=np.float32)
    token_position = np.asarray(token_position)
    Wq, Wk, Wv, Wo = (np.asarray(w, dtype=np.float32) for w in (Wq, Wk, Wv, Wo))
    bq, bk, bv = (np.asarray(b_, dtype=np.float32) for b_ in (bq, bk, bv))

    in_maps = []
    tables = {}
    for c in range(N_CORES):
        b, hg = divmod(c, HG)
        heads = range(HG * hg, HG * hg + HG)
        # permuted q/k column indices for this core's heads
        cols_qk = np.concatenate([DK * h + _PERM for h in heads])
        cols_v = np.arange(NCOLS * hg, NCOLS * hg + NCOLS)
        if b not in tables:
            tables[b] = _rope_tables(np.asarray(token_position[b]))
        cosT, sinT = tables[b]
        wo_rows = Wo[cols_v, :]  # [256, 1024]
        in_maps.append({
            "xT": to_bf16(x[b].T),                              # [1024, 2048]
            "wq": to_bf16(Wq[:, cols_qk]),                      # [1024, 256]
            "wk": to_bf16(Wk[:, cols_qk]),
            "wv": to_bf16(Wv[:, cols_v]),
            "wo": to_bf16(wo_rows.reshape(HG, DK, D).transpose(1, 0, 2)),  # [64, 4, 1024]
            "bq": to_bf16(bq[cols_qk][None, :]),                # [1, 256]
            "bk": to_bf16(bk[cols_qk][None, :]),
            "bv": to_bf16(bv[cols_v][None, :]),
            "ones_row": to_bf16(np.ones((1, SB), np.float32)),
            "onesc": to_bf16(np.ones((128, DK), np.float32)),
            "onesr": np.ones((128, DK), np.float32),
            "maskd": _CAUSAL_MASKS,
            "cosT": cosT,
            "sinT": sinT,
        })
    return in_maps


# ---------------------------------------------------------------------------
# device program
# ---------------------------------------------------------------------------

def build_program(with_bias=False):
    from concourse import bacc, library_config
    nc = bacc.Bacc("TRN2", debug=False)

    xT = nc.declare_dram_parameter("xT", [D, S], BF16, isOutput=False).ap()
    wq = nc.declare_dram_parameter("wq", [D, NCOLS], BF16, isOutput=False).ap()
    wk = nc.declare_dram_parameter("wk", [D, NCOLS], BF16, isOutput=False).ap()
    wv = nc.declare_dram_parameter("wv", [D, NCOLS], BF16, isOutput=False).ap()
    wo = nc.declare_dram_parameter("wo", [DK, HG, D], BF16, isOutput=False).ap()
    bq = nc.declare_dram_parameter("bq", [1, NCOLS], BF16, isOutput=False).ap()
    bk = nc.declare_dram_parameter("bk", [1, NCOLS], BF16, isOutput=False).ap()
    bv = nc.declare_dram_parameter("bv", [1, NCOLS], BF16, isOutput=False).ap()
    ones_row_d = nc.declare_dram_parameter("ones_row", [1, SB], BF16, isOutput=False).ap()
    onesc_d = nc.declare_dram_parameter("onesc", [128, DK], BF16, isOutput=False).ap()
    onesr_d = nc.declare_dram_parameter("onesr", [128, DK], F32R, isOutput=False).ap()
    maskd_d = nc.declare_dram_parameter("maskd", [128, 4, 2 * SB], BF16, isOutput=False).ap()
    cosT = nc.declare_dram_parameter("cosT", [128, S], F32, isOutput=False).ap()
    sinT = nc.declare_dram_parameter("sinT", [128, S], F32, isOutput=False).ap()
    out = nc.declare_dram_parameter("out", [S, D], F32, isOutput=True).ap()
    debug_dump = os.environ.get("KERNEL_DEBUG_DUMP", "0") == "1"
    if debug_dump:
        dbg_qt = nc.declare_dram_parameter("dbg_qt", [128, SB], F32, isOutput=True).ap()
        dbg_kh = nc.declare_dram_parameter("dbg_kh", [128, SB], F32, isOutput=True).ap()
        dbg_va = nc.declare_dram_parameter("dbg_va", [128, HG * (DK + 8)], F32, isOutput=True).ap()
        dbg_ot = nc.declare_dram_parameter("dbg_ot", [NSB, HG, DK, SB], F32, isOutput=True).ap()
        dbg_den = nc.declare_dram_parameter("dbg_den", [NSB, HG, 2, SB], F32, isOutput=True).ap()

    with tile.TileContext(nc) as tc, ExitStack() as ctx:
        nc.gpsimd.load_library(library_config.proxy)
        const = ctx.enter_context(tc.tile_pool(name="const", bufs=1))
        sbig = ctx.enter_context(tc.tile_pool(name="sbig", bufs=1))
        rtmp = ctx.enter_context(tc.tile_pool(name="rtmp", bufs=2))
        epool = ctx.enter_context(tc.tile_pool(name="epool", bufs=3))
        npool = ctx.enter_context(tc.tile_pool(name="npool", bufs=2))
        opool = ctx.enter_context(tc.tile_pool(name="opool", bufs=2))
        sc_ps = ctx.enter_context(tc.tile_pool(name="sc_ps", bufs=2, space="PSUM"))
        pv_ps = ctx.enter_context(tc.tile_pool(name="pv_ps", bufs=2, space="PSUM"))
        mm_ps = ctx.enter_context(tc.tile_pool(name="mm_ps", bufs=2, space="PSUM"))

        # --- static SBUF tiles
        wq_sb = [const.tile([128, NCOLS], BF16, tag=f"wq{dc}", name=f"wq{dc}")
                 for dc in range(NDC)]
        wk_sb = [const.tile([128, NCOLS], BF16, tag=f"wk{dc}", name=f"wk{dc}")
                 for dc in range(NDC)]
        wv_sb = [const.tile([128, NCOLS], BF16, tag=f"wv{dc}", name=f"wv{dc}")
                 for dc in range(NDC)]
        cos_sb = const.tile([128, S], F32, tag="cos")
        sin_sb = const.tile([128, S], F32, tag="sin")
        wo_sb = const.tile([DK, HG, D], BF16, tag="wo")
        onesc_sb = const.tile([128, DK], BF16, tag="onesc")
        onesr_sb = const.tile([128, DK], F32R, tag="onesr")
        mask_sb = const.tile([128, 4, 2 * SB], BF16, tag="maskd")
        if with_bias:
            bq_sb = const.tile([1, NCOLS], BF16, tag="bq")
            bk_sb = const.tile([1, NCOLS], BF16, tag="bk")
            bv_sb = const.tile([1, NCOLS], BF16, tag="bv")
            ones_row = const.tile([1, SB], BF16, tag="ones_row")
        xt = [[sbig.tile([128, SB], BF16, tag=f"xt{sb}_{dc}", name=f"xt{sb}_{dc}")
               for dc in range(NDC)] for sb in range(NSB)]
        # Q^T / K^T per (chunk, sq-block): chunk c holds heads {2c, 2c+1}
        # stacked on partitions (head 2c rows 0:64, head 2c+1 rows 64:128)
        qt = [[sbig.tile([128, SB], BF16, tag=f"qt{c}_{sb}", name=f"qt{c}_{sb}")
               for sb in range(NSB)] for c in range(2)]
        kth = [[sbig.tile([128, SB], BF16, tag=f"kh{c}_{sb}", name=f"kh{c}_{sb}")
                for sb in range(NSB)] for c in range(2)]
        # V augmented per key tile, every head [V(64) | one] so PV row 64
        # accumulates the softmax denominator
        vaug = [sbig.tile([128, HG * AUGW], BF16, tag=f"va{st}", name=f"va{st}")
                for st in range(NST)]
        # normalized O^T per (head, sq-block), rows 0:64
        ot = [[sbig.tile([DK, SB], BF16, tag=f"ot{h}_{j}", name=f"ot{h}_{j}")
               for j in range(NSB)] for h in range(HG)]


        # --- DMAs, critical-path first
        for dc in range(NDC):
            nc.sync.dma_start(wq_sb[dc][:], wq[128 * dc:128 * dc + 128, :])
            nc.sync.dma_start(xt[0][dc][:], xT[128 * dc:128 * dc + 128, 0:SB])
        nc.sync.dma_start(cos_sb[:], cosT)
        nc.sync.dma_start(sin_sb[:], sinT)
        for dc in range(NDC):
            nc.sync.dma_start(wk_sb[dc][:], wk[128 * dc:128 * dc + 128, :])
        nc.sync.dma_start(mask_sb[:], maskd_d)
        nc.sync.dma_start(onesc_sb[:], onesc_d)
        for dc in range(NDC):
            nc.sync.dma_start(wv_sb[dc][:], wv[128 * dc:128 * dc + 128, :])
        nc.sync.dma_start(onesr_sb[:], onesr_d)
        for dc in range(NDC):
            nc.sync.dma_start(xt[1][dc][:], xT[128 * dc:128 * dc + 128, SB:2 * SB])
        nc.sync.dma_start(wo_sb[:], wo)
        if with_bias:
            nc.sync.dma_start(bq_sb[:], bq)
            nc.sync.dma_start(bk_sb[:], bk)
            nc.sync.dma_start(bv_sb[:], bv)
            nc.sync.dma_start(ones_row[:], ones_row_d)
        for sb in (2, 3):
            for dc in range(NDC):
                nc.sync.dma_start(xt[sb][dc][:],
                                  xT[128 * dc:128 * dc + 128, SB * sb:SB * sb + SB])

        # --- emission helpers -------------------------------------------
        def emit_qk_chunk(sb, c, wname):
            """Projection chunk c of Q or K for sq block sb, incl. RoPE."""
            w_sb = wq_sb if wname == "q" else wk_sb
            ss = slice(SB * sb, SB * sb + SB)
            ncol = slice(128 * c, 128 * c + 128)
            ps = mm_ps.tile([128, SB], F32, tag="mm", name="ps_qk")
            for dc in range(NDC):
                nc.tensor.matmul(ps[:], w_sb[dc][:, ncol], xt[sb][dc][:],
                                 start=(dc == 0),
                                 stop=(dc == NDC - 1 and not with_bias))
            if with_bias:
                b_sb = bq_sb if wname == "q" else bk_sb
                nc.tensor.matmul(ps[:], b_sb[0:1, ncol], ones_row[0:1, :],
                                 start=False, stop=True)
            # rope: dst = ps*cos + shuffle(ps)*sin
            t_cos = rtmp.tile([128, SB], F32, tag="rc", name="t_cos")
            nc.vector.tensor_mul(t_cos[:], ps[:], cos_sb[:, ss])
            t_shuf = rtmp.tile([128, SB], F32, tag="rs", name="t_shuf")
            nc.vector.stream_shuffle(t_shuf[:], ps[:], SHUF_MASK)
            t_sin = rtmp.tile([128, SB], F32, tag="rm", name="t_sin")
            nc.gpsimd.tensor_mul(t_sin[:], t_shuf[:], sin_sb[:, ss])
            dst = qt[c][sb] if wname == "q" else kth[c][sb]
            nc.vector.tensor_add(dst[:], t_cos[:], t_sin[:])

        def emit_v_st(sb, st4):
            """V projection for one 128-seq tile, scattered into vaug."""
            st = 4 * sb + st4
            ps = mm_ps.tile([128, SB], F32, tag="mm", name="ps_v")
            for dc in range(NDC):
                nc.tensor.matmul(ps[:, 0:NCOLS],
                                 xt[sb][dc][:, 128 * st4:128 * st4 + 128],
                                 wv_sb[dc][:],
                                 start=(dc == 0),
                                 stop=(dc == NDC - 1 and not with_bias))
            if with_bias:
                nc.tensor.matmul(ps[:, 0:NCOLS], ones_row[0:1, 0:128],
                                 bv_sb[0:1, :], start=False, stop=True)
            va = vaug[st][:].rearrange("p (h e) -> p h e", h=HG)
            psv = ps[:, 0:NCOLS].rearrange("p (h k) -> p h k", h=HG)
            nc.vector.tensor_copy(va[:, :, 0:DK], psv[:, :, :])
            nc.vector.tensor_copy(va[:, :, DK], onesc_sb[:, 0:HG])

        wo_copy_tick = [0]

        def emit_wo(st, dc):
            """Output projection for one (128-seq, 512-dmodel) tile."""
            jb = st // 4
            rq = slice(128 * (st % 4), 128 * (st % 4) + 128)
            cols = slice(SB * dc, SB * dc + SB)
            ps = mm_ps.tile([128, SB], F32, tag="mm", name="ps_wo")
            for h in range(HG):
                nc.tensor.matmul(ps[:], ot[h][jb][:, rq], wo_sb[:, h, cols],
                                 start=(h == 0), stop=(h == HG - 1))
            o_sb = opool.tile([128, SB], F32, tag="osb", name="o_sb")
            if wo_copy_tick[0] % 2 == 0:
                nc.vector.tensor_copy(o_sb[:], ps[:])
            else:
                nc.scalar.copy(o_sb[:], ps[:])
            wo_copy_tick[0] += 1
            nc.sync.dma_start(out[128 * st:128 * st + 128, cols], o_sb[:])

        def emit_norm(pv_t, h, j):
            """ot[h][j] = pv V-rows * broadcast(1/denominator).

            The DVE reciprocal is element-serial per lane, so running it on
            the [1, 512] denominator row costs 3.3us. Instead transpose the
            row through 32x32 stream-transpose blocks so the 512 values land
            on 32 partitions (16 per lane), take the reciprocal there
            (~0.2us), and transpose back."""
            tT = npool.tile([128, SB], F32, tag="tt", name="tT")
            nc.vector.transpose(tT[DK:DK + 32, :], pv_t[DK:DK + 32, :])
            rT = npool.tile([128, SB], F32, tag="rt", name="rT")
            nc.vector.memset(rT[DK:DK + 32, :], 0.0)
            nc.vector.reciprocal(rT[DK:DK + 32, 0:SB:32],
                                 tT[DK:DK + 32, 0:SB:32])
            rec = npool.tile([128, SB], F32, tag="rec", name="rec")
            nc.vector.transpose(rec[DK:DK + 32, :], rT[DK:DK + 32, :])
            rec_b = npool.tile([128, SB], BF16, tag="recb", name="rec_b")
            nc.vector.tensor_copy(rec_b[DK:DK + 1, :], rec[DK:DK + 1, :])
            bcp = mm_ps.tile([128, SB], F32, tag="mm", name="bcp")
            nc.tensor.matmul(bcp[0:DK, :], onesc_sb[DK:DK + 1, :],
                             rec_b[DK:DK + 1, :],
                             start=True, stop=True)
            nc.vector.tensor_mul(ot[h][j][:], pv_t[0:DK, :], bcp[0:DK, :])

        # --- projections for block 0 chunk 0 (pair 1's chunks become
        # the first fillers inside pair 0's attention)
        emit_qk_chunk(0, 0, "q")
        emit_qk_chunk(0, 0, "k")
        for st4 in range(4):
            emit_v_st(0, st4)

        # --- main interleaved stream ------------------------------------
        seq = os.environ.get("KERNEL_SEQ", "0") == "1"
        if seq:
            emit_qk_chunk(0, 1, "q")
            emit_qk_chunk(0, 1, "k")
            for nb in range(1, NSB):
                emit_qk_chunk(nb, 0, "q")
                emit_qk_chunk(nb, 0, "k")
                emit_qk_chunk(nb, 1, "q")
                emit_qk_chunk(nb, 1, "k")
                for st4 in range(4):
                    emit_v_st(nb, st4)
        fillers = deque()
        if not seq:
            fillers.append(lambda: emit_qk_chunk(0, 1, "q"))
            fillers.append(lambda: emit_qk_chunk(0, 1, "k"))
        pending_norm = deque()  # closures, flushed after the next sc group
        for j in range(NSB):
            if not seq and j < NSB - 1:
                nb = j + 1
                fillers.append(lambda nb=nb: emit_qk_chunk(nb, 0, "q"))
                fillers.append(lambda nb=nb: emit_qk_chunk(nb, 0, "k"))
                fillers.append(lambda nb=nb: emit_qk_chunk(nb, 1, "q"))
                fillers.append(lambda nb=nb: emit_qk_chunk(nb, 1, "k"))
                for st4 in range(4):
                    fillers.append(lambda nb=nb, st4=st4: emit_v_st(nb, st4))
            if not seq and j > 0:
                for st in range(4 * (j - 1), 4 * j):
                    for dc in range(2):
                        fillers.append(lambda st=st, dc=dc: emit_wo(st, dc))

            # Head-pair interleave: the even and odd head of a chunk run as
            # two independent score/PV streams, doubling the PE's
            # dependency-free lookahead over the Scalar exp latency. PV runs
            # one key tile behind its scores so e(i) is always ready.
            for c in range(2):
                heads = (2 * c, 2 * c + 1)
                pvs = [pv_ps.tile([128, SB], F32, tag="pv", name="pv")
                       for _ in heads]
                for pv_t in pvs:
                    # rows 65:96 are read (as don't-care lanes) by the
                    # 32x32 transpose in emit_norm; initialize them once.
                    # Row 64 is re-zeroed by the first PV matmul's start=True.
                    nc.vector.memset(pv_t[DK:DK + 32, :], 0.0)
                ngrp = 4 * (j + 1)

                def emit_sc(i):
                    # both heads' score tiles fused in one 2-bank psum tile
                    # so a single exp (and mask) covers the step
                    sc = sc_ps.tile([128, 2, SB], F32, tag="sc", name="sc")
                    for parity in range(2):
                        rows = slice(DK * parity, DK * parity + DK)
                        nc.tensor.matmul(
                            sc[:, parity, :],
                            kth[c][i // 4][rows, 128 * (i % 4):128 * (i % 4) + 128],
                            qt[c][j][rows, :],
                            start=True, stop=True)
                    e = epool.tile([128, 2, SB], BF16, tag="e", name="e")
                    nc.scalar.activation(e[:], sc[:],
                                         mybir.ActivationFunctionType.Exp,
                                         scale=SCALE)
                    d = i - 4 * j
                    if d >= 0:  # tile touches the causal diagonal
                        nc.vector.tensor_mul(
                            e[:], e[:],
                            mask_sb[:, d, :].rearrange("p (t f) -> p t f", t=2))
                    return e

                def emit_pv(i, parity, e):
                    lhs = vaug[i][:].rearrange(
                        "p (h e) -> p h e", h=HG)[:, heads[parity], 0:DK + 1]
                    nc.tensor.matmul(
                        pvs[parity][0:DK + 1, :], lhs, e[:, parity, :],
                        start=(i == 0), stop=(i == ngrp - 1))

                prev = None
                for g in range(ngrp):
                    cur = (g, emit_sc(g))
                    while pending_norm:
                        pending_norm.popleft()()
                    if fillers:
                        fillers.popleft()()
                    if prev is not None:
                        emit_pv(prev[0], 0, prev[1])
                        emit_pv(prev[0], 1, prev[1])
                    prev = cur
                emit_pv(prev[0], 0, prev[1])
                emit_pv(prev[0], 1, prev[1])
                for parity in range(2):
                    # copy pv to SBUF so the psum bank frees after one op
                    # instead of after the whole normalize chain
                    pvc = npool.tile([128, SB], F32, tag=f"pvc{parity}",
                                     name="pvc")
                    nc.vector.tensor_copy(pvc[0:DK + 32, :],
                                          pvs[parity][0:DK + 32, :])
                    pending_norm.append(
                        lambda pv_t=pvc, hh=heads[parity], jj=j:
                            emit_norm(pv_t, hh, jj))

        # --- tail: last normalize + Wo for block 3
        while pending_norm:
            pending_norm.popleft()()
        while fillers:
            fillers.popleft()()
        wo_start = 0 if seq else 4 * (NSB - 1)
        for st in range(wo_start, 4 * NSB):
            for dc in range(2):
                emit_wo(st, dc)

        if debug_dump:
            dq = opool.tile([128, SB], F32, tag="dbg", name="dq")
            nc.vector.tensor_copy(dq[:], qt[0][1][:])
            nc.sync.dma_start(dbg_qt, dq[:])
            dk_ = opool.tile([128, SB], F32, tag="dbg", name="dk_")
            nc.vector.tensor_copy(dk_[:], kth[0][1][:])
            nc.sync.dma_start(dbg_kh, dk_[:])
            dv = opool.tile([128, HG * AUGW], F32, tag="dbgv", name="dv")
            nc.vector.tensor_copy(dv[:], vaug[4][:])
            nc.sync.dma_start(dbg_va, dv[:])
            for jj in range(NSB):
                for hh in range(HG):
                    do = opool.tile([128, SB], F32, tag="dbg", name="do")
                    nc.vector.tensor_copy(do[0:DK, :], ot[hh][jj][:])
                    nc.sync.dma_start(dbg_ot[jj, hh], do[0:DK, :])

    nc.compile()
    return nc


_CACHED_NC = {}


def _get_program(with_bias=False):
    if with_bias not in _CACHED_NC:
        _CACHED_NC[with_bias] = build_program(with_bias=with_bias)
    return _CACHED_NC[with_bias]


# ---------------------------------------------------------------------------
# entry point
# ---------------------------------------------------------------------------

def kernel(x, token_position, Wq, bq, Wk, bk, Wv, bv, Wo, bo, _results=None):
    from concourse.bass_utils import run_bass_kernel_spmd

    in_maps = make_core_inputs(x, token_position, Wq, bq, Wk, bk, Wv, bv, Wo, bo)
    if _results is None:
        with_bias = any(float(np.abs(np.asarray(v)).max()) != 0.0
                        for v in (bq, bk, bv))
        nc = _get_program(with_bias=with_bias)
        res = run_bass_kernel_spmd(nc, in_maps, list(range(N_CORES)))
        _results = [res.results[i]["out"] for i in range(N_CORES)]
    bo = np.asarray(bo, dtype=np.float32)
    out = np.empty((B, S, D), dtype=np.float32)
    for b in range(B):
        acc = _results[HG * b].astype(np.float32)
        for hg in range(1, HG):
            acc = acc + _results[HG * b + hg]
        out[b] = acc + bo[None, :]
    return out


# revision 40
# speedup vs baseline: 1.1446x; 1.0213x over previous
"""Trainium2 Bass kernel: causal multi-head attention with interleaved RoPE.

Problem shapes (hardcoded): x [2, 2048, 1024], 16 heads of dk=64.
Sharding: 8 cores = 2 batches x 4 head-groups (4 heads each). Each core
computes its head-slice Q/K/V projections, RoPE, causal attention, and a
partial output through its Wo row-slice; the host sums the 4 partials per
batch and adds bo.

RoPE trick: attention scores are invariant to any permutation of the dk
axis applied to both Q and K, so the Wq/Wk columns are permuted on the host
into a "quadrant half-split" layout where each rotation pair partner sits
exactly 16 partitions away inside the same 32-partition quadrant. The DVE
stream_shuffle (a per-quadrant 32-way permute) then produces the swapped
operand, and RoPE becomes: rot = q * cosT + shuffle(q) * sinT with
host-precomputed tables (sinT carries the sign).

Schedule: one interleaved instruction stream. Projections for seq-block
j+1 and the Wo output projection for block j-1 are emitted as PE fillers
between the score/PV matmuls of block j's attention, so the Tensor engine
never drains (stays at max p-state) while the Scalar engine runs exp.
Head pairs share one ot tile (odd head's PV lands on PSUM partitions
63:128 via the [1|V] augmented-V layout) so Wo needs 2 full-K matmuls
per tile instead of 4 half-K ones.
"""

import os
from collections import deque
from contextlib import ExitStack

import numpy as np
import ml_dtypes

import concourse.bass as bass
import concourse.mybir as mybir
import concourse.tile as tile

B, S, D, H = 2, 2048, 1024, 16
DK = D // H  # 64
HG = 4  # heads per core
NCOLS = HG * DK  # 256 columns of the projection per core
THETA = 10000.0
SCALE = 1.0 / float(np.sqrt(DK))
N_CORES = 8

F32 = mybir.dt.float32
F32R = mybir.dt.float32r
BF16 = mybir.dt.bfloat16

SB = 512            # sq block width
NSB = S // SB       # 4
NST = S // 128      # 16 key tiles / V tiles
NDC = D // 128      # 8 contraction chunks
GW = 1              # key tiles per score-psum group
AUGW = DK + 8       # V head stride padded so each head's lhsT is 16B aligned

BF = ml_dtypes.bfloat16


def to_bf16(a):
    return np.ascontiguousarray(np.asarray(a, dtype=np.float32)).astype(BF)


# ---------------------------------------------------------------------------
# host-side prep
# ---------------------------------------------------------------------------

def _rope_perm():
    """Within-head column permutation pi: new row r -> original dk index."""
    perm = np.empty(DK, dtype=np.int64)
    for r in range(DK):
        q, m = divmod(r, 32)
        if m < 16:
            perm[r] = 2 * (16 * q + m)
        else:
            perm[r] = 2 * (16 * q + m - 16) + 1
    return perm


_PERM = _rope_perm()
SHUF_MASK = list(range(16, 32)) + list(range(16))  # swap 16-halves per quadrant


def _causal_masks():
    """mask[p, d, t, f] = (f >= 128*d + p): causal keep-mask for a key tile
    at diagonal offset d, doubled along t so one multiply covers the fused
    even|odd head pair of score columns."""
    p_ = np.arange(128)[:, None, None, None]
    d_ = np.arange(4)[None, :, None, None]
    f_ = np.arange(SB)[None, None, None, :]
    keep = (f_ >= 128 * d_ + p_) | (np.arange(2)[None, None, :, None] < 0)
    keep = np.broadcast_to(keep, (128, 4, 2, SB)).reshape(128, 4, 2 * SB)
    return to_bf16(keep.astype(np.float32))


_CAUSAL_MASKS = _causal_masks()


def _rope_tables(pos):
    """cosT/sinT [128, S] fp32 for the permuted layout. pos: [S] int."""
    inv_freq = (np.float32(THETA) ** (-(np.arange(0, DK, 2, dtype=np.float32) / np.float32(DK))))  # [32]
    ang = pos.astype(np.float32)[:, None] * inv_freq[None, :]  # [S, 32]
    cos = np.cos(ang)  # [S, 32]
    sin = np.sin(ang)
    cosT = np.empty((128, S), dtype=np.float32)
    sinT = np.empty((128, S), dtype=np.float32)
    for p in range(128):
        r = p % DK
        q, m = divmod(r, 32)
        if m < 16:
            i = 16 * q + m
            sgn = -1.0
        else:
            i = 16 * q + m - 16
            sgn = 1.0
        cosT[p] = cos[:, i]
        sinT[p] = np.float32(sgn) * sin[:, i]
    return cosT, sinT


def make_core_inputs(x, token_position, Wq, bq, Wk, bk, Wv, bv, Wo, bo):
    """Build the 8 per-core input maps."""
    x = np.asarray(x, dtype=np.float32)
    token_position = np.asarray(token_position)
    Wq, Wk, Wv, Wo = (np.asarray(w, dtype=np.float32) for w in (Wq, Wk, Wv, Wo))
    bq, bk, bv = (np.asarray(b_, dtype=np.float32) for b_ in (bq, bk, bv))

    in_maps = []
    tables = {}
    for c in range(N_CORES):
        b, hg = divmod(c, HG)
        heads = range(HG * hg, HG * hg + HG)
        # permuted q/k column indices for this core's heads
        cols_qk = np.concatenate([DK * h + _PERM for h in heads])
        cols_v = np.arange(NCOLS * hg, NCOLS * hg + NCOLS)
        if b not in tables:
            tables[b] = _rope_tables(np.asarray(token_position[b]))
        cosT, sinT = tables[b]
        wo_rows = Wo[cols_v, :]  # [256, 1024]
        in_maps.append({
            "xT": to_bf16(x[b].T),                              # [1024, 2048]
            "wq": to_bf16(Wq[:, cols_qk]),                      # [1024, 256]
            "wk": to_bf16(Wk[:, cols_qk]),
            "wv": to_bf16(Wv[:, cols_v]),
            "wo": to_bf16(wo_rows.reshape(HG, DK, D).transpose(1, 0, 2)),  # [64, 4, 1024]
            "bq": to_bf16(bq[cols_qk][None, :]),                # [1, 256]
            "bk": to_bf16(bk[cols_qk][None, :]),
            "bv": to_bf16(bv[cols_v][None, :]),
            "ones_row": to_bf16(np.ones((1, SB), np.float32)),
            "onesc": to_bf16(np.ones((128, DK), np.float32)),
            "onesr": np.ones((128, DK), np.float32),
            "maskd": _CAUSAL_MASKS,
            "cosT": to_bf16(cosT),
            "sinT": to_bf16(sinT),
        })
    return in_maps


# ---------------------------------------------------------------------------
# device program
# ---------------------------------------------------------------------------

def build_program(with_bias=False):
    from concourse import bacc, library_config
    nc = bacc.Bacc("TRN2", debug=False)

    xT = nc.declare_dram_parameter("xT", [D, S], BF16, isOutput=False).ap()
    wq = nc.declare_dram_parameter("wq", [D, NCOLS], BF16, isOutput=False).ap()
    wk = nc.declare_dram_parameter("wk", [D, NCOLS], BF16, isOutput=False).ap()
    wv = nc.declare_dram_parameter("wv", [D, NCOLS], BF16, isOutput=False).ap()
    wo = nc.declare_dram_parameter("wo", [DK, HG, D], BF16, isOutput=False).ap()
    bq = nc.declare_dram_parameter("bq", [1, NCOLS], BF16, isOutput=False).ap()
    bk = nc.declare_dram_parameter("bk", [1, NCOLS], BF16, isOutput=False).ap()
    bv = nc.declare_dram_parameter("bv", [1, NCOLS], BF16, isOutput=False).ap()
    ones_row_d = nc.declare_dram_parameter("ones_row", [1, SB], BF16, isOutput=False).ap()
    onesc_d = nc.declare_dram_parameter("onesc", [128, DK], BF16, isOutput=False).ap()
    onesr_d = nc.declare_dram_parameter("onesr", [128, DK], F32R, isOutput=False).ap()
    maskd_d = nc.declare_dram_parameter("maskd", [128, 4, 2 * SB], BF16, isOutput=False).ap()
    cosT = nc.declare_dram_parameter("cosT", [128, S], BF16, isOutput=False).ap()
    sinT = nc.declare_dram_parameter("sinT", [128, S], BF16, isOutput=False).ap()
    out = nc.declare_dram_parameter("out", [S, D], F32, isOutput=True).ap()
    debug_dump = os.environ.get("KERNEL_DEBUG_DUMP", "0") == "1"
    if debug_dump:
        dbg_qt = nc.declare_dram_parameter("dbg_qt", [128, SB], F32, isOutput=True).ap()
        dbg_kh = nc.declare_dram_parameter("dbg_kh", [128, SB], F32, isOutput=True).ap()
        dbg_va = nc.declare_dram_parameter("dbg_va", [128, HG * (DK + 8)], F32, isOutput=True).ap()
        dbg_ot = nc.declare_dram_parameter("dbg_ot", [NSB, HG, DK, SB], F32, isOutput=True).ap()
        dbg_den = nc.declare_dram_parameter("dbg_den", [NSB, HG, 2, SB], F32, isOutput=True).ap()

    with tile.TileContext(nc) as tc, ExitStack() as ctx:
        nc.gpsimd.load_library(library_config.proxy)
        const = ctx.enter_context(tc.tile_pool(name="const", bufs=1))
        sbig = ctx.enter_context(tc.tile_pool(name="sbig", bufs=1))
        rtmp = ctx.enter_context(tc.tile_pool(name="rtmp", bufs=2))
        epool = ctx.enter_context(tc.tile_pool(name="epool", bufs=3))
        npool = ctx.enter_context(tc.tile_pool(name="npool", bufs=2))
        opool = ctx.enter_context(tc.tile_pool(name="opool", bufs=2))
        sc_ps = ctx.enter_context(tc.tile_pool(name="sc_ps", bufs=2, space="PSUM"))
        pv_ps = ctx.enter_context(tc.tile_pool(name="pv_ps", bufs=2, space="PSUM"))
        mm_ps = ctx.enter_context(tc.tile_pool(name="mm_ps", bufs=2, space="PSUM"))

        # --- static SBUF tiles
        wq_sb = [const.tile([128, NCOLS], BF16, tag=f"wq{dc}", name=f"wq{dc}")
                 for dc in range(NDC)]
        wk_sb = [const.tile([128, NCOLS], BF16, tag=f"wk{dc}", name=f"wk{dc}")
                 for dc in range(NDC)]
        wv_sb = [const.tile([128, NCOLS], BF16, tag=f"wv{dc}", name=f"wv{dc}")
                 for dc in range(NDC)]
        cos_sb = const.tile([128, S], F32, tag="cos")
        sin_sb = const.tile([128, S], F32, tag="sin")
        cos_bf = const.tile([128, S], BF16, tag="cosb")
        sin_bf = const.tile([128, S], BF16, tag="sinb")
        wo_sb = const.tile([DK, HG, D], BF16, tag="wo")
        onesc_sb = const.tile([128, DK], BF16, tag="onesc")
        onesr_sb = const.tile([128, DK], F32R, tag="onesr")
        mask_sb = const.tile([128, 4, 2 * SB], BF16, tag="maskd")
        if with_bias:
            bq_sb = const.tile([1, NCOLS], BF16, tag="bq")
            bk_sb = const.tile([1, NCOLS], BF16, tag="bk")
            bv_sb = const.tile([1, NCOLS], BF16, tag="bv")
            ones_row = const.tile([1, SB], BF16, tag="ones_row")
        xt = [[sbig.tile([128, SB], BF16, tag=f"xt{sb}_{dc}", name=f"xt{sb}_{dc}")
               for dc in range(NDC)] for sb in range(NSB)]
        # Q^T / K^T per (chunk, sq-block): chunk c holds heads {2c, 2c+1}
        # stacked on partitions (head 2c rows 0:64, head 2c+1 rows 64:128)
        qt = [[sbig.tile([128, SB], BF16, tag=f"qt{c}_{sb}", name=f"qt{c}_{sb}")
               for sb in range(NSB)] for c in range(2)]
        kth = [[sbig.tile([128, SB], BF16, tag=f"kh{c}_{sb}", name=f"kh{c}_{sb}")
                for sb in range(NSB)] for c in range(2)]
        # V augmented per key tile, every head [V(64) | one] so PV row 64
        # accumulates the softmax denominator
        vaug = [sbig.tile([128, HG * AUGW], BF16, tag=f"va{st}", name=f"va{st}")
                for st in range(NST)]
        # normalized O^T per (head, sq-block), rows 0:64
        ot = [[sbig.tile([DK, SB], BF16, tag=f"ot{h}_{j}", name=f"ot{h}_{j}")
               for j in range(NSB)] for h in range(HG)]
        # static staging for the denominator chain: pvc rows 65:96 and rT's
        # unwritten columns are zeroed once; heads reuse the tiles serially
        pvc_st = [sbig.tile([128, SB], F32, tag=f"pvc{par}", name=f"pvc{par}")
                  for par in range(2)]
        rT_st = sbig.tile([128, SB], F32, tag="rT_st")
        for par in range(2):
            nc.vector.memset(pvc_st[par][DK:DK + 32, :], 0.0)
        nc.vector.memset(rT_st[DK:DK + 32, :], 0.0)


        # --- DMAs, critical-path first
        for dc in range(NDC):
            nc.sync.dma_start(wq_sb[dc][:], wq[128 * dc:128 * dc + 128, :])
            nc.sync.dma_start(xt[0][dc][:], xT[128 * dc:128 * dc + 128, 0:SB])
        nc.sync.dma_start(cos_bf[:], cosT)
        nc.sync.dma_start(sin_bf[:], sinT)
        nc.vector.tensor_copy(cos_sb[:], cos_bf[:])
        nc.vector.tensor_copy(sin_sb[:], sin_bf[:])
        for dc in range(NDC):
            nc.sync.dma_start(wk_sb[dc][:], wk[128 * dc:128 * dc + 128, :])
        nc.sync.dma_start(mask_sb[:], maskd_d)
        nc.sync.dma_start(onesc_sb[:], onesc_d)
        for dc in range(NDC):
            nc.sync.dma_start(wv_sb[dc][:], wv[128 * dc:128 * dc + 128, :])
        nc.sync.dma_start(onesr_sb[:], onesr_d)
        for dc in range(NDC):
            nc.sync.dma_start(xt[1][dc][:], xT[128 * dc:128 * dc + 128, SB:2 * SB])
        nc.sync.dma_start(wo_sb[:], wo)
        if with_bias:
            nc.sync.dma_start(bq_sb[:], bq)
            nc.sync.dma_start(bk_sb[:], bk)
            nc.sync.dma_start(bv_sb[:], bv)
            nc.sync.dma_start(ones_row[:], ones_row_d)
        for sb in (2, 3):
            for dc in range(NDC):
                nc.sync.dma_start(xt[sb][dc][:],
                                  xT[128 * dc:128 * dc + 128, SB * sb:SB * sb + SB])

        # --- emission helpers -------------------------------------------
        def emit_qk_chunk(sb, c, wname):
            """Projection chunk c of Q or K for sq block sb, incl. RoPE."""
            w_sb = wq_sb if wname == "q" else wk_sb
            ss = slice(SB * sb, SB * sb + SB)
            ncol = slice(128 * c, 128 * c + 128)
            ps = mm_ps.tile([128, SB], F32, tag="mm", name="ps_qk")
            for dc in range(NDC):
                nc.tensor.matmul(ps[:], w_sb[dc][:, ncol], xt[sb][dc][:],
                                 start=(dc == 0),
                                 stop=(dc == NDC - 1 and not with_bias))
            if with_bias:
                b_sb = bq_sb if wname == "q" else bk_sb
                nc.tensor.matmul(ps[:], b_sb[0:1, ncol], ones_row[0:1, :],
                                 start=False, stop=True)
            # rope: dst = ps*cos + shuffle(ps)*sin
            t_cos = rtmp.tile([128, SB], F32, tag="rc", name="t_cos")
            nc.vector.tensor_mul(t_cos[:], ps[:], cos_sb[:, ss])
            t_shuf = rtmp.tile([128, SB], F32, tag="rs", name="t_shuf")
            nc.vector.stream_shuffle(t_shuf[:], ps[:], SHUF_MASK)
            t_sin = rtmp.tile([128, SB], F32, tag="rm", name="t_sin")
            nc.gpsimd.tensor_mul(t_sin[:], t_shuf[:], sin_sb[:, ss])
            dst = qt[c][sb] if wname == "q" else kth[c][sb]
            nc.vector.tensor_add(dst[:], t_cos[:], t_sin[:])

        def emit_v_st(sb, st4):
            """V projection for one 128-seq tile, scattered into vaug."""
            st = 4 * sb + st4
            ps = mm_ps.tile([128, SB], F32, tag="mm", name="ps_v")
            for dc in range(NDC):
                nc.tensor.matmul(ps[:, 0:NCOLS],
                                 xt[sb][dc][:, 128 * st4:128 * st4 + 128],
                                 wv_sb[dc][:],
                                 start=(dc == 0),
                                 stop=(dc == NDC - 1 and not with_bias))
            if with_bias:
                nc.tensor.matmul(ps[:, 0:NCOLS], ones_row[0:1, 0:128],
                                 bv_sb[0:1, :], start=False, stop=True)
            va = vaug[st][:].rearrange("p (h e) -> p h e", h=HG)
            psv = ps[:, 0:NCOLS].rearrange("p (h k) -> p h k", h=HG)
            nc.vector.tensor_copy(va[:, :, 0:DK], psv[:, :, :])
            nc.vector.tensor_copy(va[:, :, DK], onesc_sb[:, 0:HG])

        wo_copy_tick = [0]

        def emit_wo(st, dc):
            """Output projection for one (128-seq, 512-dmodel) tile."""
            jb = st // 4
            rq = slice(128 * (st % 4), 128 * (st % 4) + 128)
            cols = slice(SB * dc, SB * dc + SB)
            ps = mm_ps.tile([128, SB], F32, tag="mm", name="ps_wo")
            for h in range(HG):
                nc.tensor.matmul(ps[:], ot[h][jb][:, rq], wo_sb[:, h, cols],
                                 start=(h == 0), stop=(h == HG - 1))
            o_sb = opool.tile([128, SB], F32, tag="osb", name="o_sb")
            nc.vector.tensor_copy(o_sb[:], ps[:])
            nc.sync.dma_start(out[128 * st:128 * st + 128, cols], o_sb[:])

        def emit_norm(pv_t, h, j):
            """ot[h][j] = pv V-rows * broadcast(1/denominator).

            The DVE reciprocal is element-serial per lane, so running it on
            the [1, 512] denominator row costs 3.3us. Instead transpose the
            row through 32x32 stream-transpose blocks so the 512 values land
            on 32 partitions (16 per lane), take the reciprocal there
            (~0.2us), and transpose back."""
            tT = npool.tile([128, SB], F32, tag="tt", name="tT")
            nc.vector.transpose(tT[DK:DK + 32, :], pv_t[DK:DK + 32, :])
            nc.vector.reciprocal(rT_st[DK:DK + 32, 0:SB:32],
                                 tT[DK:DK + 32, 0:SB:32])
            rec = npool.tile([128, SB], F32, tag="rec", name="rec")
            nc.vector.transpose(rec[DK:DK + 32, :], rT_st[DK:DK + 32, :])
            rec_b = npool.tile([128, SB], BF16, tag="recb", name="rec_b")
            nc.vector.tensor_copy(rec_b[DK:DK + 1, :], rec[DK:DK + 1, :])
            bcp = mm_ps.tile([128, SB], F32, tag="mm", name="bcp")
            nc.tensor.matmul(bcp[0:DK, :], onesc_sb[DK:DK + 1, :],
                             rec_b[DK:DK + 1, :],
                             start=True, stop=True)
            nc.vector.tensor_mul(ot[h][j][:], pv_t[0:DK, :], bcp[0:DK, :])

        # --- projections for block 0 chunk 0 (pair 1's chunks become
        # the first fillers inside pair 0's attention)
        emit_qk_chunk(0, 0, "q")
        emit_qk_chunk(0, 0, "k")
        for st4 in range(4):
            emit_v_st(0, st4)

        # --- main interleaved stream ------------------------------------
        seq = os.environ.get("KERNEL_SEQ", "0") == "1"
        if seq:
            emit_qk_chunk(0, 1, "q")
            emit_qk_chunk(0, 1, "k")
            for nb in range(1, NSB):
                emit_qk_chunk(nb, 0, "q")
                emit_qk_chunk(nb, 0, "k")
                emit_qk_chunk(nb, 1, "q")
                emit_qk_chunk(nb, 1, "k")
                for st4 in range(4):
                    emit_v_st(nb, st4)
        fillers = deque()
        if not seq:
            fillers.append(lambda: emit_qk_chunk(0, 1, "q"))
            fillers.append(lambda: emit_qk_chunk(0, 1, "k"))
        pending_norm = deque()  # closures, flushed after the next sc group
        for j in range(NSB):
            if not seq and j < NSB - 1:
                nb = j + 1
                fillers.append(lambda nb=nb: emit_qk_chunk(nb, 0, "q"))
                fillers.append(lambda nb=nb: emit_qk_chunk(nb, 0, "k"))
                fillers.append(lambda nb=nb: emit_qk_chunk(nb, 1, "q"))
                fillers.append(lambda nb=nb: emit_qk_chunk(nb, 1, "k"))
                for st4 in range(4):
                    fillers.append(lambda nb=nb, st4=st4: emit_v_st(nb, st4))
            wo_blocks = {1: [0], 3: [1, 2]}
            if not seq:
                for jb in wo_blocks.get(j, []):
                    for st in range(4 * jb, 4 * jb + 4):
                        for dc in range(2):
                            fillers.append(
                                lambda st=st, dc=dc: emit_wo(st, dc))

            # Head-pair interleave: the even and odd head of a chunk run as
            # two independent score/PV streams, doubling the PE's
            # dependency-free lookahead over the Scalar exp latency. PV runs
            # one key tile behind its scores so e(i) is always ready.
            for c in range(2):
                heads = (2 * c, 2 * c + 1)
                pvs = [pv_ps.tile([128, SB], F32, tag="pv", name="pv")
                       for _ in heads]
                ngrp = 4 * (j + 1)

                def emit_sc(i):
                    # both heads' score tiles fused in one 2-bank psum tile
                    # so a single exp (and mask) covers the step
                    sc = sc_ps.tile([128, 2, SB], F32, tag="sc", name="sc")
                    for parity in range(2):
                        rows = slice(DK * parity, DK * parity + DK)
                        nc.tensor.matmul(
                            sc[:, parity, :],
                            kth[c][i // 4][rows, 128 * (i % 4):128 * (i % 4) + 128],
                            qt[c][j][rows, :],
                            start=True, stop=True)
                    e = epool.tile([128, 2, SB], BF16, tag="e", name="e")
                    nc.scalar.activation(e[:], sc[:],
                                         mybir.ActivationFunctionType.Exp,
                                         scale=SCALE)
                    d = i - 4 * j
                    if d >= 0:  # tile touches the causal diagonal
                        nc.vector.tensor_mul(
                            e[:], e[:],
                            mask_sb[:, d, :].rearrange("p (t f) -> p t f", t=2))
                    return e

                def emit_pv(i, parity, e):
                    lhs = vaug[i][:].rearrange(
                        "p (h e) -> p h e", h=HG)[:, heads[parity], 0:DK + 1]
                    nc.tensor.matmul(
                        pvs[parity][0:DK + 1, :], lhs, e[:, parity, :],
                        start=(i == 0), stop=(i == ngrp - 1))

                prev = None
                for g in range(ngrp):
                    cur = (g, emit_sc(g))
                    while pending_norm:
                        pending_norm.popleft()()
                    if fillers:
                        fillers.popleft()()
                    if prev is not None:
                        emit_pv(prev[0], 0, prev[1])
                        emit_pv(prev[0], 1, prev[1])
                    prev = cur
                emit_pv(prev[0], 0, prev[1])
                emit_pv(prev[0], 1, prev[1])
                for parity in range(2):
                    # copy pv to SBUF so the psum bank frees after one op
                    # instead of after the whole normalize chain
                    pvc = pvc_st[parity]
                    nc.vector.tensor_copy(pvc[0:DK + 1, :],
                                          pvs[parity][0:DK + 1, :])
                    pending_norm.append(
                        lambda pv_t=pvc, hh=heads[parity], jj=j:
                            emit_norm(pv_t, hh, jj))

        # --- tail: last normalize + Wo for block 3
        while pending_norm:
            pending_norm.popleft()()
        while fillers:
            fillers.popleft()()
        wo_start = 0 if seq else 4 * (NSB - 1)
        for st in range(wo_start, 4 * NSB):
            for dc in range(2):
                emit_wo(st, dc)

        if debug_dump:
            dq = opool.tile([128, SB], F32, tag="dbg", name="dq")
            nc.vector.tensor_copy(dq[:], qt[0][1][:])
            nc.sync.dma_start(dbg_qt, dq[:])
            dk_ = opool.tile([128, SB], F32, tag="dbg", name="dk_")
            nc.vector.tensor_copy(dk_[:], kth[0][1][:])
            nc.sync.dma_start(dbg_kh, dk_[:])
            dv = opool.tile([128, HG * AUGW], F32, tag="dbgv", name="dv")
            nc.vector.tensor_copy(dv[:], vaug[4][:])
            nc.sync.dma_start(dbg_va, dv[:])
            for jj in range(NSB):
                for hh in range(HG):
                    do = opool.tile([128, SB], F32, tag="dbg", name="do")
                    nc.vector.tensor_copy(do[0:DK, :], ot[hh][jj][:])
                    nc.sync.dma_start(dbg_ot[jj, hh], do[0:DK, :])

    nc.compile()
    return nc


_CACHED_NC = {}


def _get_program(with_bias=False):
    if with_bias not in _CACHED_NC:
        _CACHED_NC[with_bias] = build_program(with_bias=with_bias)
    return _CACHED_NC[with_bias]


# ---------------------------------------------------------------------------
# entry point
# ---------------------------------------------------------------------------

def kernel(x, token_position, Wq, bq, Wk, bk, Wv, bv, Wo, bo, _results=None):
    from concourse.bass_utils import run_bass_kernel_spmd

    in_maps = make_core_inputs(x, token_position, Wq, bq, Wk, bk, Wv, bv, Wo, bo)
    if _results is None:
        with_bias = any(float(np.abs(np.asarray(v)).max()) != 0.0
                        for v in (bq, bk, bv))
        nc = _get_program(with_bias=with_bias)
        res = run_bass_kernel_spmd(nc, in_maps, list(range(N_CORES)))
        _results = [res.results[i]["out"] for i in range(N_CORES)]
    bo = np.asarray(bo, dtype=np.float32)
    out = np.empty((B, S, D), dtype=np.float32)
    for b in range(B):
        acc = _results[HG * b].astype(np.float32)
        for hg in range(1, HG):
            acc = acc + _results[HG * b + hg]
        out[b] = acc + bo[None, :]
    return out
